# revision 1
# baseline (speedup 1.0000x reference)
"""Trainium2 Bass kernel for nn_DLUPack (CARAFE-style dynamic upsampling).

Sharding: 8 cores = (batch n in [0,4)) x (output-row-parity s in {0,1});
core (n, s) computes low-res rows hh in [32s, 32s+32) -> all parity-s output rows.

Reference output mapping (its reshape scrambles positions):
  ref[n, c, 2y+i, 2x+j] = sum_k patches[c, hh, ww, k] * kern[hh, ww, k, u]
  with hh = 32s + 16jh + m:  row r = 8m + 2(ww//16) + s, col = 8*(ww%16) + 2u + jh.

Device pipeline per core:
  1. compressor 1x1 conv (PE, bf16) -> cx [64, 38, 66]
  2. offset+mask 3x3 convs (9 accumulated MMs) -> psum [57, .]: off ch 0-7, mask ch 32-56
  3. exp in ACT evac, PE-transpose -> expT [64 w, 36 g, 25 k], softmax via free-dim reduce
  4. offset PE-transpose -> deltas; indicator bilinear weights W9 (DVE)
  5. kernc assembly: 9 broadcast-multiply terms (stride-0 APs) + adds (DVE)
  6. kernc -> bf16; 4 partition-shifted variants via SBUF-SBUF DMA
  7. per pair m: 10 data-prep copies -> 2 local_scatter (GPSIMD) -> banded [128, 5x512]
  8. carafe: 5 accumulated MMs [128,128]x[128,512] per (pair, c-half) -> psum [128,512]
  9. ACT evac -> DMA out 4 contiguous output rows
"""
import sys
import numpy as np

sys.path.insert(0, '/opt/trn_rl_repo')

import ml_dtypes  # noqa: E402
from contextlib import ExitStack  # noqa: E402

import concourse.bass as bass  # noqa: E402
import concourse.tile as tile  # noqa: E402
from concourse import mybir, bacc  # noqa: E402
from concourse.bass_utils import run_bass_kernel_spmd  # noqa: E402

F32 = mybir.dt.float32
BF16 = mybir.dt.float16  # NOTE: fp16 (better mantissa), name kept for brevity
I16 = mybir.dt.int16
AF = mybir.ActivationFunctionType
OP = mybir.AluOpType

N, C, H, W = 4, 256, 64, 64


def _ap(base, off_elems, dims):
    return bass.AP(tensor=base.tensor, offset=base.offset + off_elems, ap=[list(d) for d in dims])


def build_scatter_tables():
    idx1 = -np.ones((128, 100), np.int16)
    idx2 = -np.ones((128, 100), np.int16)
    for p in range(128):
        jh, wpp = p // 64, p % 64
        for b in range(5):
            w = wpp + b - 2
            if not (0 <= w < 64):
                continue
            q, wl = w // 16, w % 16
            for ki in range(5):
                for u in range(4):
                    col = q * 128 + 8 * wl + 2 * u + jh
                    qidx = (b * 5 + ki) * 4 + u
                    if ki < 3:
                        idx1[p, qidx] = ki * 512 + col
                    else:
                        idx2[p, qidx] = (ki - 3) * 512 + col
    return idx1, idx2


def build_program():
    nc = bacc.Bacc(None, target_bir_lowering=False, debug=True)

    xwin = nc.declare_dram_parameter('xwin', [2, 128, 38 * 64], BF16, isOutput=False)
    xT2 = nc.declare_dram_parameter('xT2', [128, 20 * 256], BF16, isOutput=False)
    wc = nc.declare_dram_parameter('wc', [128, 2 * 64], BF16, isOutput=False)
    wk = nc.declare_dram_parameter('wk', [64, 9 * 57], BF16, isOutput=False)
    bco = nc.declare_dram_parameter('bco', [57, 1], F32, isOutput=False)
    bcomp = nc.declare_dram_parameter('bcomp', [64, 1], F32, isOutput=False)
    wvec = nc.declare_dram_parameter('wvec', [64, 1], F32, isOutput=False)
    w63 = nc.declare_dram_parameter('w63', [64, 1], F32, isOutput=False)
    hrow = nc.declare_dram_parameter('hrow', [64, 32], F32, isOutput=False)
    y63 = nc.declare_dram_parameter('y63', [64, 32], F32, isOutput=False)
    ident = nc.declare_dram_parameter('ident', [128, 128], F32, isOutput=False)
    idx1 = nc.declare_dram_parameter('idx1', [128, 100], I16, isOutput=False)
    idx2 = nc.declare_dram_parameter('idx2', [128, 100], I16, isOutput=False)
    zed = nc.declare_dram_parameter('zed', [2, 3600], BF16, isOutput=False)
    outp = nc.declare_dram_parameter('outp', [256, 64 * 128], F32, isOutput=True)

    with tile.TileContext(nc) as tc, ExitStack() as ctx:
        sing = ctx.enter_context(tc.tile_pool(name='sing', bufs=1))
        work = ctx.enter_context(tc.tile_pool(name='work', bufs=1))
        loop = ctx.enter_context(tc.tile_pool(name='loop', bufs=3))
        band = ctx.enter_context(tc.tile_pool(name='band', bufs=4))
        rowp = ctx.enter_context(tc.tile_pool(name='rowp', bufs=4))
        psum = ctx.enter_context(tc.psum_pool(name='ps', bufs=2))
        psc = ctx.enter_context(tc.psum_pool(name='psc', bufs=3))

        def load(shape, dtype, src):
            t = sing.tile(shape, dtype, name=f'ld_{src.tensor.name if hasattr(src, "tensor") else id(src)}')
            nc.sync.dma_start(out=t[:], in_=src[:])
            return t

        xwin_sb = sing.tile([128, 2, 38 * 64], BF16)
        for cg_ in range(2):
            nc.sync.dma_start(out=xwin_sb[:, cg_, :],
                              in_=_ap(xwin[:], cg_ * 128 * 2432, [[2432, 128], [1, 2432]]))
        xT2_sb = load([128, 20 * 256], BF16, xT2)
        wc_sb = load([128, 2, 64], BF16, wc)
        wk_sb = load([64, 9, 57], BF16, wk)
        bco_sb = load([57, 1], F32, bco)
        bcomp_sb = load([64, 1], F32, bcomp)
        wvec_sb = load([64, 1], F32, wvec)
        w63_sb = load([64, 1], F32, w63)
        hrow_sb = load([64, 32], F32, hrow)
        y63_sb = load([64, 32], F32, y63)
        id_sb = load([128, 128], F32, ident)
        idx1_sb = load([128, 100], I16, idx1)
        idx2_sb = load([128, 100], I16, idx2)

        # PE warm-up: keep TensorE busy during input-DMA wait so HAM reaches 8/8
        pw = psc.tile([128, 512], F32, name='pcs_warm', tag='pcs')
        for _ in range(90):
            nc.tensor.matmul(pw[0:64, 0:64], id_sb[:, 0:64], id_sb[:, 0:64], start=True, stop=True)

        # hoisted variant buffers; edge partitions zeroed via tiny DMAs from DRAM zeros
        msm4_p1 = work.tile([64, 36 * 100], BF16)
        msm4_m1 = work.tile([64, 36 * 100], BF16)
        nc.sync.dma_start(out=_ap(msm4_p1[:], 63 * 3600, [[3600, 1], [1, 3600]]),
                          in_=_ap(zed[:], 0, [[3600, 1], [1, 3600]]))
        nc.scalar.dma_start(out=_ap(msm4_m1[:], 0, [[3600, 1], [1, 3600]]),
                            in_=_ap(zed[:], 0, [[3600, 1], [1, 3600]]))
        kbf_sh = {}
        for d in (-2, -1, 1, 2):
            kbf_sh[d] = work.tile([64, 3200], BF16, name=f'kbf{d}')
            eng = nc.sync if d > 0 else nc.scalar
            if d > 0:
                eng.dma_start(out=_ap(kbf_sh[d][:], (64 - d) * 3200, [[3200, d], [1, 3200]]),
                              in_=_ap(zed[:], 0, [[3600, d], [1, 3200]]))
            else:
                eng.dma_start(out=_ap(kbf_sh[d][:], 0, [[3200, -d], [1, 3200]]),
                              in_=_ap(zed[:], 0, [[3600, -d], [1, 3200]]))

        # ---- 1. compressor ----
        cx_sb = work.tile([64, 38, 66], BF16)
        nc.vector.memset(_ap(cx_sb[:], 0, [[38 * 66, 64], [66, 38], [1, 1]]), 0.0)
        nc.vector.memset(_ap(cx_sb[:], 65, [[38 * 66, 64], [66, 38], [1, 1]]), 0.0)
        for grp in range(5):
            g0 = grp * 8
            rows = min(8, 38 - g0)
            nn = rows * 64
            pcs = psum.tile([64, 512], F32)
            for cg in range(2):
                nc.tensor.matmul(pcs[:, :nn], wc_sb[:, cg, :],
                                 xwin_sb[:, cg, g0 * 64:g0 * 64 + nn],
                                 start=(cg == 0), stop=(cg == 1))
            nc.scalar.activation(
                out=_ap(cx_sb[:], g0 * 66 + 1, [[38 * 66, 64], [66, rows], [1, 64]]),
                in_=_ap(pcs[:], 0, [[512, 64], [64, rows], [1, 64]]),
                func=AF.Identity, bias=bcomp_sb[:], scale=1.0)

        # ---- 2. offset+mask convs ----
        expS = work.tile([25, 36, 64], F32)
        offS = work.tile([8, 32, 64], F32)
        for grp in range(6):
            g0 = grp * 6
            nn = 6 * 64
            pcs = psum.tile([57, 384], F32)
            for t in range(9):
                dy, dx = t // 3, t % 3
                rhs = _ap(cx_sb[:], (g0 + dy) * 66 + dx, [[38 * 66, 64], [66, 6], [1, 64]])
                nc.tensor.matmul(pcs[:, :nn], wk_sb[:, t, :], rhs,
                                 start=(t == 0), stop=(t == 8))
            nc.scalar.activation(out=expS[:, g0:g0 + 6, :],
                                 in_=_ap(pcs[:], 32 * 384, [[384, 25], [64, 6], [1, 64]]),
                                 func=AF.Exp, bias=bco_sb[32:57], scale=1.0)
            lo, hi = max(g0, 2), min(g0 + 6, 34)
            if lo < hi:
                nc.vector.tensor_scalar(
                    out=offS[:, lo - 2:hi - 2, :],
                    in0=_ap(pcs[:], (lo - g0) * 64, [[384, 8], [64, hi - lo], [1, 64]]),
                    scalar1=bco_sb[0:8], scalar2=None, op0=OP.add)

        # ---- 3. transpose exp -> expT; softmax (pair-batched transposes) ----
        expT = work.tile([64, 36, 25], F32)
        for half in range(2):
            pcnt = 10 if half == 0 else 8      # g-pairs this bank
            pt = psum.tile([128, 512], F32)
            for i in range(pcnt):
                g = (half * 10 + i) * 2
                # in [25, 128] = rows (g, g+1) -> out [128, 25]
                nc.tensor.transpose(pt[:, i * 25:i * 25 + 25],
                                    _ap(expS[:], g * 64, [[36 * 64, 25], [1, 128]]),
                                    id_sb[0:25, 0:25])
            for sub in range(2):
                nc.scalar.activation(
                    out=_ap(expT[:], (half * 20 + sub) * 25,
                            [[900, 64], [50, pcnt], [1, 25]]),
                    in_=_ap(pt[:], sub * 64 * 512, [[512, 64], [25, pcnt], [1, 25]]),
                    func=AF.Copy, scale=1.0)
        sumT = work.tile([64, 36], F32)
        nc.vector.tensor_reduce(out=sumT[:], in_=expT[:], axis=mybir.AxisListType.X, op=OP.add)
        recT = work.tile([64, 36], F32)
        nc.vector.reciprocal(out=recT[:], in_=sumT[:])
        msm = work.tile([64, 36, 25], F32)
        nc.vector.tensor_tensor(out=msm[:], in0=expT[:],
                                in1=_ap(recT[:], 0, [[36, 64], [1, 36], [0, 25]]), op=OP.mult)
        # partition-shifted variants via SBUF-SBUF DMA (edges zero)
        msm_p1 = work.tile([64, 36, 25], F32)   # msm_p1[p] = msm[p+1]
        msm_m1 = work.tile([64, 36, 25], F32)   # msm_m1[p] = msm[p-1]
        nc.vector.memset(msm_p1[:], 0.0)
        nc.vector.memset(msm_m1[:], 0.0)
        nc.sync.dma_start(out=_ap(msm_p1[:], 0, [[900, 63], [1, 900]]),
                          in_=_ap(msm[:], 900, [[900, 63], [1, 900]]))
        nc.sync.dma_start(out=_ap(msm_m1[:], 900, [[900, 63], [1, 900]]),
                          in_=_ap(msm[:], 0, [[900, 63], [1, 900]]))

        # ---- 4. offset transpose; W9 ----
        deltT = work.tile([64, 32, 8], BF16)
        po_t = psc.tile([128, 512], F32, name='po_w', tag='pcs')
        po = po_t[:, 0:128]
        for i in range(16):
            nc.tensor.transpose(po_t[:, i * 8:i * 8 + 8],
                                _ap(offS[:], i * 128, [[32 * 64, 8], [1, 128]]),
                                id_sb[0:8, 0:8])
        for sub in range(2):
            nc.scalar.activation(
                out=_ap(deltT[:], sub * 8, [[256, 64], [16, 16], [1, 8]]),
                in_=_ap(po_t[:], sub * 64 * 512, [[512, 64], [8, 16], [1, 8]]),
                func=AF.Copy, scale=1.0)

        def dview(chbase):
            return _ap(deltT[:], chbase, [[256, 64], [8, 32], [1, 4]])

        def wt(nm):
            return work.tile([64, 128], BF16, name=nm)

        t1, t2 = wt('t1'), wt('t2')
        gxc, x0r, wxt, omwx, x1r = wt('gxc'), wt('x0r'), wt('wxt'), wt('omwx'), wt('x1r')
        gyc, y0r, wyt, omwy, y1r = wt('gyc'), wt('y0r'), wt('wyt'), wt('omwy'), wt('y1r')
        ia, ib = wt('ia'), wt('ib')
        cwx = work.tile([64, 3, 128], BF16)
        rwy = work.tile([64, 3, 128], BF16)
        W9b = work.tile([64, 9 * 128], BF16)


        hrow_bc = _ap(hrow_sb[:], 0, [[32, 64], [1, 32], [0, 4]])
        y63_bc = _ap(y63_sb[:], 0, [[32, 64], [1, 32], [0, 4]])

        def r4(ap):
            return _ap(ap, 0, [[128, 64], [4, 32], [1, 4]])

        nc.vector.tensor_scalar(out=t1[:], in0=dview(0), scalar1=wvec_sb[:], scalar2=None, op0=OP.add)
        nc.vector.tensor_scalar(out=t2[:], in0=t1[:], scalar1=0.0, scalar2=63.0, op0=OP.max, op1=OP.min)
        nc.vector.tensor_scalar(out=gxc[:], in0=t2[:], scalar1=wvec_sb[:], scalar2=None, op0=OP.subtract)
        nc.vector.tensor_scalar(out=x0r[:], in0=gxc[:], scalar1=0.0, scalar2=-1.0, op0=OP.is_lt, op1=OP.mult)
        nc.vector.tensor_tensor(out=wxt[:], in0=gxc[:], in1=x0r[:], op=OP.subtract)
        nc.vector.tensor_scalar(out=omwx[:], in0=wxt[:], scalar1=-1.0, scalar2=1.0, op0=OP.mult, op1=OP.add)
        nc.vector.tensor_scalar(out=x1r[:], in0=x0r[:], scalar1=1.0, scalar2=w63_sb[:], op0=OP.add, op1=OP.min)

        nc.vector.tensor_tensor(out=r4(t1[:]), in0=dview(4), in1=hrow_bc, op=OP.add)
        nc.vector.tensor_scalar(out=t2[:], in0=t1[:], scalar1=0.0, scalar2=63.0, op0=OP.max, op1=OP.min)
        nc.vector.tensor_tensor(out=r4(gyc[:]), in0=r4(t2[:]), in1=hrow_bc, op=OP.subtract)
        nc.vector.tensor_scalar(out=y0r[:], in0=gyc[:], scalar1=0.0, scalar2=-1.0, op0=OP.is_lt, op1=OP.mult)
        nc.vector.tensor_tensor(out=wyt[:], in0=gyc[:], in1=y0r[:], op=OP.subtract)
        nc.vector.tensor_scalar(out=omwy[:], in0=wyt[:], scalar1=-1.0, scalar2=1.0, op0=OP.mult, op1=OP.add)
        nc.vector.tensor_scalar(out=t1[:], in0=y0r[:], scalar1=1.0, scalar2=None, op0=OP.add)
        nc.vector.tensor_tensor(out=r4(y1r[:]), in0=r4(t1[:]), in1=y63_bc, op=OP.min)

        for i, e in enumerate((-1.0, 0.0, 1.0)):
            nc.vector.tensor_scalar(out=ia[:], in0=x0r[:], scalar1=e, scalar2=None, op0=OP.is_equal)
            nc.vector.tensor_scalar(out=ib[:], in0=x1r[:], scalar1=e, scalar2=None, op0=OP.is_equal)
            nc.vector.tensor_tensor(out=ia[:], in0=ia[:], in1=omwx[:], op=OP.mult)
            nc.vector.tensor_tensor(out=ib[:], in0=ib[:], in1=wxt[:], op=OP.mult)
            nc.vector.tensor_tensor(out=cwx[:, i, :], in0=ia[:], in1=ib[:], op=OP.add)
            nc.vector.tensor_scalar(out=ia[:], in0=y0r[:], scalar1=e, scalar2=None, op0=OP.is_equal)
            nc.vector.tensor_scalar(out=ib[:], in0=y1r[:], scalar1=e, scalar2=None, op0=OP.is_equal)
            nc.vector.tensor_tensor(out=ia[:], in0=ia[:], in1=omwy[:], op=OP.mult)
            nc.vector.tensor_tensor(out=ib[:], in0=ib[:], in1=wyt[:], op=OP.mult)
            nc.vector.tensor_tensor(out=rwy[:, i, :], in0=ia[:], in1=ib[:], op=OP.add)
        for iy in range(3):
            for ix in range(3):
                nc.vector.tensor_tensor(
                    out=_ap(W9b[:], (iy * 3 + ix) * 128, [[9 * 128, 64], [1, 128]]),
                    in0=rwy[:, iy, :], in1=cwx[:, ix, :], op=OP.mult)

        # ---- 5. kernc assembly, fp16, layout [64, (h:32, k:25, u:4)] ----
        msm4 = work.tile([64, 36, 25, 4], BF16)
        nc.vector.tensor_tensor(
            out=msm4[:],
            in0=_ap(expT[:], 0, [[900, 64], [25, 36], [1, 25], [0, 4]]),
            in1=_ap(recT[:], 0, [[36, 64], [1, 36], [0, 25], [0, 4]]), op=OP.mult)
        nc.sync.dma_start(out=_ap(msm4_p1[:], 0, [[3600, 63], [1, 3600]]),
                          in_=_ap(msm4[:], 3600, [[3600, 63], [1, 3600]]))
        nc.scalar.dma_start(out=_ap(msm4_m1[:], 3600, [[3600, 63], [1, 3600]]),
                          in_=_ap(msm4[:], 0, [[3600, 63], [1, 3600]]))
        kernc = work.tile([64, 3200], BF16)
        tmp = work.tile([64, 3200], BF16)
        msm_by_ex = {-1: msm4_m1, 0: msm4, 1: msm4_p1}
        kbf = {0: kernc}
        kbf.update(kbf_sh)
        data_all = work.tile([128, 16, 100], BF16)

        # ---- 5-9 software-pipelined by m-groups ----
        NG = 4
        GM = 16 // NG

        def emit_asm(G):
            for jh in range(2):
                hofs = (16 * jh + GM * G) * 100
                kv = _ap(kernc[:], hofs, [[3200, 64], [100, GM], [4, 25], [1, 4]])
                tv = _ap(tmp[:], hofs, [[3200, 64], [100, GM], [4, 25], [1, 4]])
                first = True
                for iy, ey in enumerate((-1, 0, 1)):
                    for ix, ex in enumerate((-1, 0, 1)):
                        mv = _ap(msm_by_ex[ex][:], (2 + ey + 16 * jh + GM * G) * 100,
                                 [[3600, 64], [100, GM], [4, 25], [1, 4]])
                        wv = _ap(W9b[:], (iy * 3 + ix) * 128 + (16 * jh + GM * G) * 4,
                                 [[9 * 128, 64], [4, GM], [0, 25], [1, 4]])
                        if first:
                            nc.vector.tensor_tensor(out=kv, in0=wv, in1=mv, op=OP.mult)
                            first = False
                        else:
                            nc.vector.tensor_tensor(out=tv, in0=wv, in1=mv, op=OP.mult)
                            nc.vector.tensor_tensor(out=kv, in0=kv, in1=tv, op=OP.add)

        def emit_dmas(G):
            for d in (-2, -1, 1, 2):
                cnt = 64 - abs(d)
                eng = nc.sync
                if d > 0:
                    eng.dma_start(
                        out=_ap(kbf[d][:], GM * G * 100, [[3200, cnt], [1600, 2], [1, GM * 100]]),
                        in_=_ap(kbf[0][:], d * 3200 + GM * G * 100,
                                [[3200, cnt], [1600, 2], [1, GM * 100]]))
                else:
                    eng.dma_start(
                        out=_ap(kbf[d][:], -d * 3200 + GM * G * 100,
                                [[3200, cnt], [1600, 2], [1, GM * 100]]),
                        in_=_ap(kbf[0][:], GM * G * 100, [[3200, cnt], [1600, 2], [1, GM * 100]]))

        def emit_prep(G):
            for jh in range(2):
                for b in range(5):
                    nc.vector.tensor_copy(
                        out=_ap(data_all[:], 64 * jh * 1600 + GM * G * 100 + b * 20,
                                [[1600, 64], [100, GM], [4, 5], [1, 4]]),
                        in_=_ap(kbf[b - 2][:], (16 * jh + GM * G) * 100 + (4 - b) * 4,
                                [[3200, 64], [100, GM], [20, 5], [1, 4]]))

        def emit_pairs(G):
            for m in range(GM * G, GM * G + GM):
                banded1 = band.tile([128, 1536], BF16, name=f'band1_{m}', tag='band1')
                banded2 = band.tile([128, 1024], BF16, name=f'band2_{m}', tag='band2')
                nc.gpsimd.local_scatter(out_ap=banded1[:], data_ap=data_all[:, m, :],
                                        idxs_ap=idx1_sb[:], channels=128, num_elems=1536, num_idxs=100)
                nc.gpsimd.local_scatter(out_ap=banded2[:], data_ap=data_all[:, m, :],
                                        idxs_ap=idx2_sb[:], channels=128, num_elems=1024, num_idxs=100)
                for ch in range(2):
                    pcs = psc.tile([128, 512], F32, name=f'pcs_{m}_{ch}', tag='pcs')
                    for ki in range(5):
                        lhsT = _ap(xT2_sb[:], (m + ki) * 256 + ch * 128, [[20 * 256, 128], [1, 128]])
                        rhs = banded1[:, ki * 512:ki * 512 + 512] if ki < 3 \
                            else banded2[:, (ki - 3) * 512:(ki - 3) * 512 + 512]
                        nc.tensor.matmul(pcs[:], lhsT, rhs, start=(ki == 0), stop=(ki == 4))
                    rb = rowp.tile([128, 512], F32, name=f'rb_{m}_{ch}', tag='rb')
                    nc.scalar.activation(out=rb[:], in_=pcs[:], func=AF.Copy, scale=1.0)
                    nc.scalar.dma_start(
                        out=_ap(outp[:], ch * 128 * 8192 + 4 * m * 128,
                                [[8192, 128], [128, 4], [1, 128]]),
                        in_=rb[:])

        emit_asm(0)
        emit_dmas(0)
        for G in range(NG):
            if G + 1 < NG:
                emit_asm(G + 1)
                emit_dmas(G + 1)
            emit_prep(G)
            emit_pairs(G)
    nc.finalize()
    return nc


_PROGRAM = None
_SCAT = build_scatter_tables()


def _get_program():
    global _PROGRAM
    if _PROGRAM is None:
        _PROGRAM = build_program()
    return _PROGRAM


def _prep_core_inputs(inputs, n, s):
    bf = np.float16
    x = np.asarray(inputs['x'][n], np.float32)
    h0 = 32 * s
    xw = np.zeros((C, 38, W), np.float32)
    for i, g in enumerate(range(h0 - 3, h0 + 35)):
        if 0 <= g < H:
            xw[:, i] = x[:, g]
    xwin = np.ascontiguousarray(xw.reshape(2, 128, 38 * 64)).astype(bf)
    xT2 = np.zeros((128, 20, C), np.float32)
    for jh in range(2):
        base = h0 + 16 * jh - 2
        for i in range(20):
            g = base + i
            if 0 <= g < H:
                xT2[64 * jh:64 * jh + 64, i] = x[:, g].T
    xT2 = np.ascontiguousarray(xT2.reshape(128, 20 * 256)).astype(bf)
    w_comp = np.asarray(inputs['w_comp'], np.float32)[:, :, 0, 0]
    wc = np.zeros((2, 128, 64), np.float32)
    for cg in range(2):
        wc[cg] = w_comp[:, cg * 128:(cg + 1) * 128].T
    wc = np.ascontiguousarray(wc.transpose(1, 0, 2).reshape(128, 2 * 64)).astype(bf)
    w_ker = np.asarray(inputs['w_ker'], np.float32)
    w_off = np.asarray(inputs['w_off'], np.float32)
    wk = np.zeros((9, 64, 57), np.float32)
    for t in range(9):
        wk[t, :, 0:8] = w_off[:, :, t // 3, t % 3].T
        wk[t, :, 32:57] = w_ker[:, :, t // 3, t % 3].T
    wk = np.ascontiguousarray(wk.transpose(1, 0, 2).reshape(64, 9 * 57)).astype(bf)
    bcov = np.zeros((57, 1), np.float32)
    bcov[0:8, 0] = np.asarray(inputs['b_off'], np.float32)
    bcov[32:57, 0] = np.asarray(inputs['b_ker'], np.float32)
    idx1, idx2 = _SCAT
    hr = (h0 + np.arange(32, dtype=np.float32))[None, :].repeat(64, 0)
    return {
        'xwin': xwin, 'xT2': xT2, 'wc': wc, 'wk': wk, 'bco': bcov,
        'bcomp': np.asarray(inputs['b_comp'], np.float32).reshape(64, 1),
        'wvec': np.arange(64, dtype=np.float32).reshape(64, 1),
        'w63': (63 - np.arange(64, dtype=np.float32)).reshape(64, 1),
        'hrow': np.ascontiguousarray(hr),
        'y63': np.ascontiguousarray(63.0 - hr),
        'ident': np.eye(128, dtype=np.float32),
        'idx1': idx1, 'idx2': idx2,
        'zed': np.zeros((2, 3600), np.float16),
    }


def kernel(**inputs):
    nc = _get_program()
    core_ids = list(range(8))
    in_maps = [_prep_core_inputs(inputs, cid // 2, cid % 2) for cid in core_ids]
    res = run_bass_kernel_spmd(nc, in_maps, core_ids)
    out = np.zeros((N, C, 128, 128), np.float32)
    for cid in core_ids:
        n, s = cid // 2, cid % 2
        op = np.asarray(res.results[cid]['outp']).reshape(256, 64, 128)
        out[n, :, s::2] = op
    return out


if __name__ == '__main__':
    d = np.load('/root/problem/ref_io.npz')
    inp = {k: d[k] for k in ('x', 'w_comp', 'b_comp', 'w_ker', 'b_ker', 'w_off', 'b_off')}
    out = kernel(**inp)
    ref = d['out']
    err = np.abs(out - ref).max()
    print('max abs err:', err, 'rel:', err / np.abs(ref).max())



# revision 11
# speedup vs baseline: 1.0285x; 1.0285x over previous
"""Trainium2 Bass kernel for nn_DLUPack (CARAFE-style dynamic upsampling).

Sharding: 8 cores = (batch n in [0,4)) x (output-row-parity s in {0,1});
core (n, s) computes low-res rows hh in [32s, 32s+32) -> all parity-s output rows.

Reference output mapping (its reshape scrambles positions):
  ref[n, c, 2y+i, 2x+j] = sum_k patches[c, hh, ww, k] * kern[hh, ww, k, u]
  with hh = 32s + 16jh + m:  row r = 8m + 2(ww//16) + s, col = 8*(ww%16) + 2u + jh.

Device pipeline per core:
  1. compressor 1x1 conv (PE, bf16) -> cx [64, 38, 66]
  2. offset+mask 3x3 convs (9 accumulated MMs) -> psum [57, .]: off ch 0-7, mask ch 32-56
  3. exp in ACT evac, PE-transpose -> expT [64 w, 36 g, 25 k], softmax via free-dim reduce
  4. offset PE-transpose -> deltas; indicator bilinear weights W9 (DVE)
  5. kernc assembly: 9 broadcast-multiply terms (stride-0 APs) + adds (DVE)
  6. kernc -> bf16; 4 partition-shifted variants via SBUF-SBUF DMA
  7. per pair m: 10 data-prep copies -> 2 local_scatter (GPSIMD) -> banded [128, 5x512]
  8. carafe: 5 accumulated MMs [128,128]x[128,512] per (pair, c-half) -> psum [128,512]
  9. ACT evac -> DMA out 4 contiguous output rows
"""
import sys
import numpy as np

sys.path.insert(0, '/opt/trn_rl_repo')

import ml_dtypes  # noqa: E402
from contextlib import ExitStack  # noqa: E402

import concourse.bass as bass  # noqa: E402
import concourse.tile as tile  # noqa: E402
from concourse import mybir, bacc  # noqa: E402
from concourse.bass_utils import run_bass_kernel_spmd  # noqa: E402

F32 = mybir.dt.float32
BF16 = mybir.dt.float16  # NOTE: fp16 (better mantissa), name kept for brevity
I16 = mybir.dt.int16
AF = mybir.ActivationFunctionType
OP = mybir.AluOpType

N, C, H, W = 4, 256, 64, 64


def _ap(base, off_elems, dims):
    return bass.AP(tensor=base.tensor, offset=base.offset + off_elems, ap=[list(d) for d in dims])


def build_scatter_tables():
    idx1 = -np.ones((128, 100), np.int16)
    idx2 = -np.ones((128, 100), np.int16)
    for p in range(128):
        jh, wpp = p // 64, p % 64
        for b in range(5):
            w = wpp + b - 2
            if not (0 <= w < 64):
                continue
            q, wl = w // 16, w % 16
            for ki in range(5):
                for u in range(4):
                    col = q * 128 + 8 * wl + 2 * u + jh
                    qidx = (b * 5 + ki) * 4 + u
                    if ki < 3:
                        idx1[p, qidx] = ki * 512 + col
                    else:
                        idx2[p, qidx] = (ki - 3) * 512 + col
    return idx1, idx2


def build_program():
    nc = bacc.Bacc(None, target_bir_lowering=False, debug=True)

    xwin = nc.declare_dram_parameter('xwin', [2, 128, 38 * 64], BF16, isOutput=False)
    xT2 = nc.declare_dram_parameter('xT2', [128, 20 * 256], BF16, isOutput=False)
    wc = nc.declare_dram_parameter('wc', [128, 2 * 64], BF16, isOutput=False)
    wk = nc.declare_dram_parameter('wk', [64, 9 * 57], BF16, isOutput=False)
    bco = nc.declare_dram_parameter('bco', [57, 1], F32, isOutput=False)
    bcomp = nc.declare_dram_parameter('bcomp', [64, 1], F32, isOutput=False)
    wvec = nc.declare_dram_parameter('wvec', [64, 1], F32, isOutput=False)
    w63 = nc.declare_dram_parameter('w63', [64, 1], F32, isOutput=False)
    hrow = nc.declare_dram_parameter('hrow', [64, 32], F32, isOutput=False)
    y63 = nc.declare_dram_parameter('y63', [64, 32], F32, isOutput=False)
    ident = nc.declare_dram_parameter('ident', [128, 128], F32, isOutput=False)
    idx1 = nc.declare_dram_parameter('idx1', [128, 100], I16, isOutput=False)
    idx2 = nc.declare_dram_parameter('idx2', [128, 100], I16, isOutput=False)
    zed = nc.declare_dram_parameter('zed', [2, 3600], BF16, isOutput=False)
    outp = nc.declare_dram_parameter('outp', [256, 64 * 128], BF16, isOutput=True)

    with tile.TileContext(nc) as tc, ExitStack() as ctx:
        sing = ctx.enter_context(tc.tile_pool(name='sing', bufs=1))
        work = ctx.enter_context(tc.tile_pool(name='work', bufs=1))
        loop = ctx.enter_context(tc.tile_pool(name='loop', bufs=3))
        band = ctx.enter_context(tc.tile_pool(name='band', bufs=4))
        rowp = ctx.enter_context(tc.tile_pool(name='rowp', bufs=4))
        psum = ctx.enter_context(tc.psum_pool(name='ps', bufs=2))
        psc = ctx.enter_context(tc.psum_pool(name='psc', bufs=3))

        def load(shape, dtype, src):
            t = sing.tile(shape, dtype, name=f'ld_{src.tensor.name if hasattr(src, "tensor") else id(src)}')
            nc.sync.dma_start(out=t[:], in_=src[:])
            return t

        xwin_sb = sing.tile([128, 2, 38 * 64], BF16)
        for cg_ in range(2):
            nc.sync.dma_start(out=xwin_sb[:, cg_, :],
                              in_=_ap(xwin[:], cg_ * 128 * 2432, [[2432, 128], [1, 2432]]))
        xT2_sb = load([128, 20 * 256], BF16, xT2)
        wc_sb = load([128, 2, 64], BF16, wc)
        wk_sb = load([64, 9, 57], BF16, wk)
        bco_sb = load([57, 1], F32, bco)
        bcomp_sb = load([64, 1], F32, bcomp)
        wvec_sb = load([64, 1], F32, wvec)
        w63_sb = load([64, 1], F32, w63)
        hrow_sb = load([64, 32], F32, hrow)
        y63_sb = load([64, 32], F32, y63)
        id_sb = load([128, 128], F32, ident)
        idx1_sb = load([128, 100], I16, idx1)
        idx2_sb = load([128, 100], I16, idx2)

        # PE warm-up: keep TensorE busy during input-DMA wait so HAM reaches 8/8
        pw = psc.tile([128, 512], F32, name='pcs_warm', tag='pcs')
        for _ in range(90):
            nc.tensor.matmul(pw[0:64, 0:64], id_sb[:, 0:64], id_sb[:, 0:64], start=True, stop=True)

        # hoisted variant buffers; edge partitions zeroed via tiny DMAs from DRAM zeros
        msm4_p1 = work.tile([64, 36 * 100], BF16)
        msm4_m1 = work.tile([64, 36 * 100], BF16)
        nc.sync.dma_start(out=_ap(msm4_p1[:], 63 * 3600, [[3600, 1], [1, 3600]]),
                          in_=_ap(zed[:], 0, [[3600, 1], [1, 3600]]))
        nc.scalar.dma_start(out=_ap(msm4_m1[:], 0, [[3600, 1], [1, 3600]]),
                            in_=_ap(zed[:], 0, [[3600, 1], [1, 3600]]))

        # ---- 1. compressor ----
        cx_sb = work.tile([64, 38, 66], BF16)
        nc.vector.memset(_ap(cx_sb[:], 0, [[38 * 66, 64], [66, 38], [1, 1]]), 0.0)
        nc.vector.memset(_ap(cx_sb[:], 65, [[38 * 66, 64], [66, 38], [1, 1]]), 0.0)
        for grp in range(5):
            g0 = grp * 8
            rows = min(8, 38 - g0)
            nn = rows * 64
            pcs = psum.tile([64, 512], F32)
            for cg in range(2):
                nc.tensor.matmul(pcs[:, :nn], wc_sb[:, cg, :],
                                 xwin_sb[:, cg, g0 * 64:g0 * 64 + nn],
                                 start=(cg == 0), stop=(cg == 1))
            nc.scalar.activation(
                out=_ap(cx_sb[:], g0 * 66 + 1, [[38 * 66, 64], [66, rows], [1, 64]]),
                in_=_ap(pcs[:], 0, [[512, 64], [64, rows], [1, 64]]),
                func=AF.Identity, bias=bcomp_sb[:], scale=1.0)

        # ---- 2. offset+mask convs ----
        expS = work.tile([25, 36, 64], F32)
        offS = work.tile([8, 32, 64], F32)
        for grp in range(6):
            g0 = grp * 6
            nn = 6 * 64
            pcs = psum.tile([57, 384], F32)
            for t in range(9):
                dy, dx = t // 3, t % 3
                rhs = _ap(cx_sb[:], (g0 + dy) * 66 + dx, [[38 * 66, 64], [66, 6], [1, 64]])
                nc.tensor.matmul(pcs[:, :nn], wk_sb[:, t, :], rhs,
                                 start=(t == 0), stop=(t == 8))
            nc.scalar.activation(out=expS[:, g0:g0 + 6, :],
                                 in_=_ap(pcs[:], 32 * 384, [[384, 25], [64, 6], [1, 64]]),
                                 func=AF.Exp, bias=bco_sb[32:57], scale=1.0)
            lo, hi = max(g0, 2), min(g0 + 6, 34)
            if lo < hi:
                nc.vector.tensor_scalar(
                    out=offS[:, lo - 2:hi - 2, :],
                    in0=_ap(pcs[:], (lo - g0) * 64, [[384, 8], [64, hi - lo], [1, 64]]),
                    scalar1=bco_sb[0:8], scalar2=None, op0=OP.add)

        # ---- 3. transpose exp -> expT; softmax (pair-batched transposes) ----
        expT = work.tile([64, 36, 25], F32)
        for half in range(2):
            pcnt = 10 if half == 0 else 8      # g-pairs this bank
            pt = psum.tile([128, 512], F32)
            for i in range(pcnt):
                g = (half * 10 + i) * 2
                # in [25, 128] = rows (g, g+1) -> out [128, 25]
                nc.tensor.transpose(pt[:, i * 25:i * 25 + 25],
                                    _ap(expS[:], g * 64, [[36 * 64, 25], [1, 128]]),
                                    id_sb[0:25, 0:25])
            for sub in range(2):
                nc.scalar.activation(
                    out=_ap(expT[:], (half * 20 + sub) * 25,
                            [[900, 64], [50, pcnt], [1, 25]]),
                    in_=_ap(pt[:], sub * 64 * 512, [[512, 64], [25, pcnt], [1, 25]]),
                    func=AF.Copy, scale=1.0)
        sumT = work.tile([64, 36], F32)
        nc.vector.tensor_reduce(out=sumT[:], in_=expT[:], axis=mybir.AxisListType.X, op=OP.add)
        recT = work.tile([64, 36], F32)
        nc.vector.reciprocal(out=recT[:], in_=sumT[:])
        msm = work.tile([64, 36, 25], F32)
        nc.vector.tensor_tensor(out=msm[:], in0=expT[:],
                                in1=_ap(recT[:], 0, [[36, 64], [1, 36], [0, 25]]), op=OP.mult)
        # partition-shifted variants via SBUF-SBUF DMA (edges zero)
        msm_p1 = work.tile([64, 36, 25], F32)   # msm_p1[p] = msm[p+1]
        msm_m1 = work.tile([64, 36, 25], F32)   # msm_m1[p] = msm[p-1]
        nc.vector.memset(msm_p1[:], 0.0)
        nc.vector.memset(msm_m1[:], 0.0)
        nc.sync.dma_start(out=_ap(msm_p1[:], 0, [[900, 63], [1, 900]]),
                          in_=_ap(msm[:], 900, [[900, 63], [1, 900]]))
        nc.sync.dma_start(out=_ap(msm_m1[:], 900, [[900, 63], [1, 900]]),
                          in_=_ap(msm[:], 0, [[900, 63], [1, 900]]))

        # ---- 4. offset transpose; W9 ----
        deltT = work.tile([64, 32, 8], BF16)
        po_t = psc.tile([128, 512], F32, name='po_w', tag='pcs')
        po = po_t[:, 0:128]
        for i in range(16):
            nc.tensor.transpose(po_t[:, i * 8:i * 8 + 8],
                                _ap(offS[:], i * 128, [[32 * 64, 8], [1, 128]]),
                                id_sb[0:8, 0:8])
        for sub in range(2):
            nc.scalar.activation(
                out=_ap(deltT[:], sub * 8, [[256, 64], [16, 16], [1, 8]]),
                in_=_ap(po_t[:], sub * 64 * 512, [[512, 64], [8, 16], [1, 8]]),
                func=AF.Copy, scale=1.0)

        def dview(chbase):
            return _ap(deltT[:], chbase, [[256, 64], [8, 32], [1, 4]])

        def wt(nm):
            return work.tile([64, 128], BF16, name=nm)

        t1, t2 = wt('t1'), wt('t2')
        gxc, x0r, wxt, omwx, x1r = wt('gxc'), wt('x0r'), wt('wxt'), wt('omwx'), wt('x1r')
        gyc, y0r, wyt, omwy, y1r = wt('gyc'), wt('y0r'), wt('wyt'), wt('omwy'), wt('y1r')
        ia, ib = wt('ia'), wt('ib')
        cwx = work.tile([64, 3, 128], BF16)
        rwy = work.tile([64, 3, 128], BF16)
        W9b = work.tile([64, 9 * 128], BF16)


        hrow_bc = _ap(hrow_sb[:], 0, [[32, 64], [1, 32], [0, 4]])
        y63_bc = _ap(y63_sb[:], 0, [[32, 64], [1, 32], [0, 4]])

        def r4(ap):
            return _ap(ap, 0, [[128, 64], [4, 32], [1, 4]])

        nc.vector.tensor_scalar(out=t1[:], in0=dview(0), scalar1=wvec_sb[:], scalar2=None, op0=OP.add)
        nc.vector.tensor_scalar(out=t2[:], in0=t1[:], scalar1=0.0, scalar2=63.0, op0=OP.max, op1=OP.min)
        nc.vector.tensor_scalar(out=gxc[:], in0=t2[:], scalar1=wvec_sb[:], scalar2=None, op0=OP.subtract)
        nc.vector.tensor_scalar(out=x0r[:], in0=gxc[:], scalar1=0.0, scalar2=-1.0, op0=OP.is_lt, op1=OP.mult)
        nc.vector.tensor_tensor(out=wxt[:], in0=gxc[:], in1=x0r[:], op=OP.subtract)
        nc.vector.tensor_scalar(out=omwx[:], in0=wxt[:], scalar1=-1.0, scalar2=1.0, op0=OP.mult, op1=OP.add)
        nc.vector.tensor_scalar(out=x1r[:], in0=x0r[:], scalar1=1.0, scalar2=w63_sb[:], op0=OP.add, op1=OP.min)

        nc.vector.tensor_tensor(out=r4(t1[:]), in0=dview(4), in1=hrow_bc, op=OP.add)
        nc.vector.tensor_scalar(out=t2[:], in0=t1[:], scalar1=0.0, scalar2=63.0, op0=OP.max, op1=OP.min)
        nc.vector.tensor_tensor(out=r4(gyc[:]), in0=r4(t2[:]), in1=hrow_bc, op=OP.subtract)
        nc.vector.tensor_scalar(out=y0r[:], in0=gyc[:], scalar1=0.0, scalar2=-1.0, op0=OP.is_lt, op1=OP.mult)
        nc.vector.tensor_tensor(out=wyt[:], in0=gyc[:], in1=y0r[:], op=OP.subtract)
        nc.vector.tensor_scalar(out=omwy[:], in0=wyt[:], scalar1=-1.0, scalar2=1.0, op0=OP.mult, op1=OP.add)
        nc.vector.tensor_scalar(out=t1[:], in0=y0r[:], scalar1=1.0, scalar2=None, op0=OP.add)
        nc.vector.tensor_tensor(out=r4(y1r[:]), in0=r4(t1[:]), in1=y63_bc, op=OP.min)

        for i, e in enumerate((-1.0, 0.0, 1.0)):
            nc.vector.tensor_scalar(out=ia[:], in0=x0r[:], scalar1=e, scalar2=None, op0=OP.is_equal)
            nc.vector.tensor_scalar(out=ib[:], in0=x1r[:], scalar1=e, scalar2=None, op0=OP.is_equal)
            nc.vector.tensor_tensor(out=ia[:], in0=ia[:], in1=omwx[:], op=OP.mult)
            nc.vector.tensor_tensor(out=ib[:], in0=ib[:], in1=wxt[:], op=OP.mult)
            nc.vector.tensor_tensor(out=cwx[:, i, :], in0=ia[:], in1=ib[:], op=OP.add)
            nc.vector.tensor_scalar(out=ia[:], in0=y0r[:], scalar1=e, scalar2=None, op0=OP.is_equal)
            nc.vector.tensor_scalar(out=ib[:], in0=y1r[:], scalar1=e, scalar2=None, op0=OP.is_equal)
            nc.vector.tensor_tensor(out=ia[:], in0=ia[:], in1=omwy[:], op=OP.mult)
            nc.vector.tensor_tensor(out=ib[:], in0=ib[:], in1=wyt[:], op=OP.mult)
            nc.vector.tensor_tensor(out=rwy[:, i, :], in0=ia[:], in1=ib[:], op=OP.add)
        for iy in range(3):
            for ix in range(3):
                nc.vector.tensor_tensor(
                    out=_ap(W9b[:], (iy * 3 + ix) * 128, [[9 * 128, 64], [1, 128]]),
                    in0=rwy[:, iy, :], in1=cwx[:, ix, :], op=OP.mult)

        # ---- 5. kernc assembly, fp16, layout [64, (h:32, k:25, u:4)] ----
        msm4 = work.tile([64, 36, 25, 4], BF16)
        nc.vector.tensor_tensor(
            out=msm4[:],
            in0=_ap(expT[:], 0, [[900, 64], [25, 36], [1, 25], [0, 4]]),
            in1=_ap(recT[:], 0, [[36, 64], [1, 36], [0, 25], [0, 4]]), op=OP.mult)
        nc.sync.dma_start(out=_ap(msm4_p1[:], 0, [[3600, 63], [1, 3600]]),
                          in_=_ap(msm4[:], 3600, [[3600, 63], [1, 3600]]))
        nc.scalar.dma_start(out=_ap(msm4_m1[:], 3600, [[3600, 63], [1, 3600]]),
                          in_=_ap(msm4[:], 0, [[3600, 63], [1, 3600]]))
        kernc = work.tile([64, 3200], BF16)
        tmp = work.tile([64, 3200], BF16)
        msm_by_ex = {-1: msm4_m1, 0: msm4, 1: msm4_p1}
        data_all = work.tile([128, 16, 100], BF16)
        nc.vector.memset(data_all[:], 0.0)

        # ---- 5-9 software-pipelined by m-groups ----
        NG = 4
        GM = 16 // NG

        def emit_asm(G):
            for jh in range(2):
                hofs = (16 * jh + GM * G) * 100
                kv = _ap(kernc[:], hofs, [[3200, 64], [100, GM], [4, 25], [1, 4]])
                tv = _ap(tmp[:], hofs, [[3200, 64], [100, GM], [4, 25], [1, 4]])
                first = True
                for iy, ey in enumerate((-1, 0, 1)):
                    for ix, ex in enumerate((-1, 0, 1)):
                        mv = _ap(msm_by_ex[ex][:], (2 + ey + 16 * jh + GM * G) * 100,
                                 [[3600, 64], [100, GM], [4, 25], [1, 4]])
                        wv = _ap(W9b[:], (iy * 3 + ix) * 128 + (16 * jh + GM * G) * 4,
                                 [[9 * 128, 64], [4, GM], [0, 25], [1, 4]])
                        if first:
                            nc.vector.tensor_tensor(out=kv, in0=wv, in1=mv, op=OP.mult)
                            first = False
                        else:
                            nc.vector.tensor_tensor(out=tv, in0=wv, in1=mv, op=OP.mult)
                            nc.vector.tensor_tensor(out=kv, in0=kv, in1=tv, op=OP.add)

        def emit_dmas(G):
            # kernc[p+d, 16jh+m, kx=4-b, ky, u] -> data_all[64jh+p, m, b*20+ky*4+u]
            # edge partitions (p+d out of range) skipped; scatter tables have -1 there.
            for bi, b in enumerate((0, 1, 2, 3, 4)):
                d = b - 2
                cnt = 64 - abs(d)
                for jh in range(2):
                    eng = nc.sync if (bi + jh) % 2 == 0 else nc.scalar
                    eng.dma_start(
                        out=_ap(data_all[:],
                                (64 * jh + max(0, -d)) * 1600 + GM * G * 100 + b * 20,
                                [[1600, cnt], [100, GM], [1, 20]]),
                        in_=_ap(kernc[:],
                                max(0, d) * 3200 + (16 * jh + GM * G) * 100 + (4 - b) * 20,
                                [[3200, cnt], [100, GM], [1, 20]]))

        def emit_pairs(G):
            for m in range(GM * G, GM * G + GM):
                banded1 = band.tile([128, 1536], BF16, name=f'band1_{m}', tag='band1')
                banded2 = band.tile([128, 1024], BF16, name=f'band2_{m}', tag='band2')
                nc.gpsimd.local_scatter(out_ap=banded1[:], data_ap=data_all[:, m, :],
                                        idxs_ap=idx1_sb[:], channels=128, num_elems=1536, num_idxs=100)
                nc.gpsimd.local_scatter(out_ap=banded2[:], data_ap=data_all[:, m, :],
                                        idxs_ap=idx2_sb[:], channels=128, num_elems=1024, num_idxs=100)
                for ch in range(2):
                    pcs = psc.tile([128, 512], F32, name=f'pcs_{m}_{ch}', tag='pcs')
                    for ki in range(5):
                        lhsT = _ap(xT2_sb[:], (m + ki) * 256 + ch * 128, [[20 * 256, 128], [1, 128]])
                        rhs = banded1[:, ki * 512:ki * 512 + 512] if ki < 3 \
                            else banded2[:, (ki - 3) * 512:(ki - 3) * 512 + 512]
                        nc.tensor.matmul(pcs[:], lhsT, rhs, start=(ki == 0), stop=(ki == 4))
                    rb = rowp.tile([128, 512], BF16, name=f'rb_{m}_{ch}', tag='rb')
                    nc.scalar.activation(out=rb[:], in_=pcs[:], func=AF.Copy, scale=1.0)
                    nc.sync.dma_start(
                        out=_ap(outp[:], ch * 128 * 8192 + 4 * m * 128,
                                [[8192, 128], [128, 4], [1, 128]]),
                        in_=rb[:])

        emit_asm(0)
        emit_dmas(0)
        for G in range(NG):
            if G + 1 < NG:
                emit_asm(G + 1)
                emit_dmas(G + 1)
            emit_pairs(G)
    nc.finalize()
    return nc


_PROGRAM = None
_SCAT = build_scatter_tables()


def _get_program():
    global _PROGRAM
    if _PROGRAM is None:
        _PROGRAM = build_program()
    return _PROGRAM


def _prep_core_inputs(inputs, n, s):
    bf = np.float16
    x = np.asarray(inputs['x'][n], np.float32)
    h0 = 32 * s
    xw = np.zeros((C, 38, W), np.float32)
    for i, g in enumerate(range(h0 - 3, h0 + 35)):
        if 0 <= g < H:
            xw[:, i] = x[:, g]
    xwin = np.ascontiguousarray(xw.reshape(2, 128, 38 * 64)).astype(bf)
    xT2 = np.zeros((128, 20, C), np.float32)
    for jh in range(2):
        base = h0 + 16 * jh - 2
        for i in range(20):
            g = base + i
            if 0 <= g < H:
                xT2[64 * jh:64 * jh + 64, i] = x[:, g].T
    xT2 = np.ascontiguousarray(xT2.reshape(128, 20 * 256)).astype(bf)
    w_comp = np.asarray(inputs['w_comp'], np.float32)[:, :, 0, 0]
    wc = np.zeros((2, 128, 64), np.float32)
    for cg in range(2):
        wc[cg] = w_comp[:, cg * 128:(cg + 1) * 128].T
    wc = np.ascontiguousarray(wc.transpose(1, 0, 2).reshape(128, 2 * 64)).astype(bf)
    w_ker = np.asarray(inputs['w_ker'], np.float32)
    w_off = np.asarray(inputs['w_off'], np.float32)
    # mask channels permuted kx-major: new k = kx*5+ky holds w_ker[ky*5+kx]
    kperm = np.array([(k % 5) * 5 + k // 5 for k in range(25)])
    wk = np.zeros((9, 64, 57), np.float32)
    for t in range(9):
        wk[t, :, 0:8] = w_off[:, :, t // 3, t % 3].T
        wk[t, :, 32:57] = w_ker[kperm, :, t // 3, t % 3].T
    wk = np.ascontiguousarray(wk.transpose(1, 0, 2).reshape(64, 9 * 57)).astype(bf)
    bcov = np.zeros((57, 1), np.float32)
    bcov[0:8, 0] = np.asarray(inputs['b_off'], np.float32)
    bcov[32:57, 0] = np.asarray(inputs['b_ker'], np.float32)[kperm]
    idx1, idx2 = _SCAT
    hr = (h0 + np.arange(32, dtype=np.float32))[None, :].repeat(64, 0)
    return {
        'xwin': xwin, 'xT2': xT2, 'wc': wc, 'wk': wk, 'bco': bcov,
        'bcomp': np.asarray(inputs['b_comp'], np.float32).reshape(64, 1),
        'wvec': np.arange(64, dtype=np.float32).reshape(64, 1),
        'w63': (63 - np.arange(64, dtype=np.float32)).reshape(64, 1),
        'hrow': np.ascontiguousarray(hr),
        'y63': np.ascontiguousarray(63.0 - hr),
        'ident': np.eye(128, dtype=np.float32),
        'idx1': idx1, 'idx2': idx2,
        'zed': np.zeros((2, 3600), np.float16),
    }


def kernel(**inputs):
    nc = _get_program()
    core_ids = list(range(8))
    in_maps = [_prep_core_inputs(inputs, cid // 2, cid % 2) for cid in core_ids]
    res = run_bass_kernel_spmd(nc, in_maps, core_ids)
    out = np.zeros((N, C, 128, 128), np.float32)
    for cid in core_ids:
        n, s = cid // 2, cid % 2
        op = np.asarray(res.results[cid]['outp']).reshape(256, 64, 128)
        out[n, :, s::2] = op
    return out


if __name__ == '__main__':
    d = np.load('/root/problem/ref_io.npz')
    inp = {k: d[k] for k in ('x', 'w_comp', 'b_comp', 'w_ker', 'b_ker', 'w_off', 'b_off')}
    out = kernel(**inp)
    ref = d['out']
    err = np.abs(out - ref).max()
    print('max abs err:', err, 'rel:', err / np.abs(ref).max())



# revision 21
# speedup vs baseline: 1.1323x; 1.1009x over previous
"""Trainium2 Bass kernel for nn_DLUPack (CARAFE-style dynamic upsampling).

Sharding: 8 cores = (batch n in [0,4)) x (output-row-parity s in {0,1});
core (n, s) computes low-res rows hh in [32s, 32s+32) -> all parity-s output rows.

Reference output mapping (its reshape scrambles positions):
  ref[n, c, 2y+i, 2x+j] = sum_k patches[c, hh, ww, k] * kern[hh, ww, k, u]
  with hh = 32s + 16jh + m:  row r = 8m + 2(ww//16) + s, col = 8*(ww%16) + 2u + jh.

Device pipeline per core (128-partition mid-section: p = 64*jh + w):
  1. compressor 1x1 conv (PE, fp16) -> cx [64, 38, 66]
  2. offset+mask 3x3 convs (9 accumulated MMs) -> psum [57, .]: off ch 0-7, mask ch 32-56
  3. exp in ACT evac -> expS [25, 36, 64]; PE-transpose row-pairs (r, r+16)
     -> expT2 [128 = jh*64+w, 20 hl, 25 k]; softmax via free-dim reduce
  4. offset PE-transpose pairs (m, m+16) -> deltT2 [128, 16, 8]; W9 weights (DVE)
  5. kernc assembly [128, 16m, 25k, 4u]: 9 broadcast-multiply terms (DVE)
  6. kernc block-shifted +-1,+-2 via SBUF-SBUF DMA (800B runs); prep slices -> data_all
  7. per pair m: 2 local_scatter (GPSIMD) -> banded [128, 3*512 + 2*512]
  8. carafe: 5 accumulated MMs [128,128]x[128,512] per (pair, c-half) -> psum [128,512]
  9. ACT evac (fp16) -> DMA out 4 contiguous output rows (host converts to f32)
"""
import sys
import numpy as np

sys.path.insert(0, '/opt/trn_rl_repo')

import ml_dtypes  # noqa: E402
from contextlib import ExitStack  # noqa: E402

import concourse.bass as bass  # noqa: E402
import concourse.tile as tile  # noqa: E402
from concourse import mybir, bacc  # noqa: E402
from concourse.bass_utils import run_bass_kernel_spmd  # noqa: E402

F32 = mybir.dt.float32
BF16 = mybir.dt.float16  # NOTE: fp16 (better mantissa), name kept for brevity
I16 = mybir.dt.int16
AF = mybir.ActivationFunctionType
OP = mybir.AluOpType

N, C, H, W = 4, 256, 64, 64


def _ap(base, off_elems, dims):
    return bass.AP(tensor=base.tensor, offset=base.offset + off_elems, ap=[list(d) for d in dims])


def build_scatter_tables():
    idx1 = -np.ones((128, 100), np.int16)
    idx2 = -np.ones((128, 100), np.int16)
    for p in range(128):
        jh, wpp = p // 64, p % 64
        for b in range(5):
            w = wpp + b - 2
            if not (0 <= w < 64):
                continue
            q, wl = w // 16, w % 16
            for ki in range(5):
                for u in range(4):
                    col = q * 128 + 8 * wl + 2 * u + jh
                    qidx = (b * 5 + ki) * 4 + u
                    if ki < 3:
                        idx1[p, qidx] = ki * 512 + col
                    else:
                        idx2[p, qidx] = (ki - 3) * 512 + col
    return idx1, idx2


def build_program():
    nc = bacc.Bacc(None, target_bir_lowering=False, debug=True)

    xwin = nc.declare_dram_parameter('xwin', [2, 128, 38 * 64], BF16, isOutput=False)
    xT2 = nc.declare_dram_parameter('xT2', [128, 20 * 256], BF16, isOutput=False)
    wc = nc.declare_dram_parameter('wc', [128, 2 * 64], BF16, isOutput=False)
    wk = nc.declare_dram_parameter('wk', [64, 9 * 57], BF16, isOutput=False)
    bco = nc.declare_dram_parameter('bco', [57, 1], F32, isOutput=False)
    bcomp = nc.declare_dram_parameter('bcomp', [64, 1], F32, isOutput=False)
    wvec = nc.declare_dram_parameter('wvec', [128, 1], F32, isOutput=False)
    w63 = nc.declare_dram_parameter('w63', [128, 1], F32, isOutput=False)
    hrow = nc.declare_dram_parameter('hrow', [128, 16], F32, isOutput=False)
    y63 = nc.declare_dram_parameter('y63', [128, 16], F32, isOutput=False)
    ident = nc.declare_dram_parameter('ident', [128, 128], F32, isOutput=False)
    idx1 = nc.declare_dram_parameter('idx1', [128, 100], I16, isOutput=False)
    idx2 = nc.declare_dram_parameter('idx2', [128, 100], I16, isOutput=False)
    zed = nc.declare_dram_parameter('zed', [2, 3600], BF16, isOutput=False)
    outp = nc.declare_dram_parameter('outp', [256, 64 * 128], BF16, isOutput=True)

    with tile.TileContext(nc) as tc, ExitStack() as ctx:
        sing = ctx.enter_context(tc.tile_pool(name='sing', bufs=1))
        work = ctx.enter_context(tc.tile_pool(name='work', bufs=1))
        band = ctx.enter_context(tc.tile_pool(name='band', bufs=4))
        rowp = ctx.enter_context(tc.tile_pool(name='rowp', bufs=4))
        psum = ctx.enter_context(tc.psum_pool(name='ps', bufs=2))
        psc = ctx.enter_context(tc.psum_pool(name='psc', bufs=3))

        def load(shape, dtype, src, eng):
            t = sing.tile(shape, dtype, name=f'ld_{src.tensor.name if hasattr(src, "tensor") else id(src)}')
            eng.dma_start(out=t[:], in_=src[:])
            return t

        # critical small loads first so warm-up + compressor can begin
        id_sb = load([128, 128], F32, ident, nc.sync)
        wc_sb = load([128, 2, 64], BF16, wc, nc.sync)
        xwin_sb = sing.tile([128, 2, 38 * 64], BF16)
        for grp in range(5):
            g0 = grp * 8
            rows = min(8, 38 - g0)
            for cg_ in range(2):
                nc.sync.dma_start(
                    out=_ap(xwin_sb[:], cg_ * 2432 + g0 * 64,
                            [[4864, 128], [1, rows * 64]]),
                    in_=_ap(xwin[:], cg_ * 128 * 2432 + g0 * 64,
                            [[2432, 128], [1, rows * 64]]))
        wk_sb = load([64, 9, 57], BF16, wk, nc.scalar)
        xT2_sb = load([128, 20 * 256], BF16, xT2, nc.scalar)
        bco_sb = load([57, 1], F32, bco, nc.gpsimd)
        bcomp_sb = load([64, 1], F32, bcomp, nc.gpsimd)
        wvec_sb = load([128, 1], F32, wvec, nc.gpsimd)
        w63_sb = load([128, 1], F32, w63, nc.gpsimd)
        hrow_sb = load([128, 16], F32, hrow, nc.gpsimd)
        y63_sb = load([128, 16], F32, y63, nc.gpsimd)
        idx1_sb = load([128, 100], I16, idx1, nc.gpsimd)
        idx2_sb = load([128, 100], I16, idx2, nc.gpsimd)

        # msm4 shifted variants; block-edge partitions zeroed via tiny DMAs from DRAM zeros
        msm4_p1 = work.tile([128, 2000], BF16)   # msm4_p1[p] = msm4[p+1] (within 64-block)
        msm4_m1 = work.tile([128, 2000], BF16)   # msm4_m1[p] = msm4[p-1]
        for blk in range(2):
            nc.scalar.dma_start(out=_ap(msm4_p1[:], (64 * blk + 63) * 2000, [[2000, 1], [1, 2000]]),
                                in_=_ap(zed[:], 0, [[3600, 1], [1, 2000]]))
            nc.scalar.dma_start(out=_ap(msm4_m1[:], 64 * blk * 2000, [[2000, 1], [1, 2000]]),
                                in_=_ap(zed[:], 0, [[3600, 1], [1, 2000]]))

        # kernc block-shift buffers; memset once so block-edge partitions stay zero
        kbf = {}
        for d in (-2, -1, 1, 2):
            kbf[d] = work.tile([128, 1600], BF16, name=f'kbf{d}')
            nc.vector.memset(kbf[d][:], 0.0)

        # PE warm-up: keep TensorE busy during input-DMA wait (DVFS ramp)
        pw = psc.tile([128, 512], F32, name='pcs_warm', tag='pcs')
        for _ in range(60):
            nc.tensor.matmul(pw[0:64, 0:64], id_sb[:, 0:64], id_sb[:, 0:64], start=True, stop=True)

        # ---- 1. compressor ----
        cx_sb = work.tile([64, 38, 66], BF16)
        nc.vector.memset(_ap(cx_sb[:], 0, [[38 * 66, 64], [66, 38], [1, 1]]), 0.0)
        nc.vector.memset(_ap(cx_sb[:], 65, [[38 * 66, 64], [66, 38], [1, 1]]), 0.0)
        for grp in range(5):
            g0 = grp * 8
            rows = min(8, 38 - g0)
            nn = rows * 64
            pcs = psum.tile([64, 512], F32)
            for cg in range(2):
                nc.tensor.matmul(pcs[:, :nn], wc_sb[:, cg, :],
                                 xwin_sb[:, cg, g0 * 64:g0 * 64 + nn],
                                 start=(cg == 0), stop=(cg == 1))
            nc.scalar.activation(
                out=_ap(cx_sb[:], g0 * 66 + 1, [[38 * 66, 64], [66, rows], [1, 64]]),
                in_=_ap(pcs[:], 0, [[512, 64], [64, rows], [1, 64]]),
                func=AF.Identity, bias=bcomp_sb[:], scale=1.0)

        # ---- 2. offset+mask convs ----
        # expS2/offS2 store interleaved row pairs: slot 2r = row r, slot 2r+1 = row r+16
        # so the [*,128] PE transposes read contiguous pairs. expS rows 16-19 duplicated.
        expS = work.tile([25, 40, 64], F32)
        offS = work.tile([8, 32, 64], F32)

        def eslot(row):  # primary slot for mask row
            return 2 * row if row < 20 else 2 * (row - 16) + 1

        def oslot(row):  # slot for offset row (no duplication)
            return 2 * row if row < 16 else 2 * (row - 16) + 1

        for grp in range(6):
            g0 = grp * 6
            nn = 6 * 64
            pcs = psum.tile([57, 384], F32)
            for t in range(9):
                dy, dx = t // 3, t % 3
                rhs = _ap(cx_sb[:], (g0 + dy) * 66 + dx, [[38 * 66, 64], [66, 6], [1, 64]])
                nc.tensor.matmul(pcs[:, :nn], wk_sb[:, t, :], rhs,
                                 start=(t == 0), stop=(t == 8))
            # evac mask rows to interleaved slots, one ACT per maximal stride-2 run
            pairs = []
            for row in range(g0, g0 + 6):
                if row < 20:
                    pairs.append((row, 2 * row))
                if row >= 16:
                    pairs.append((row, 2 * (row - 16) + 1))
            pairs.sort(key=lambda rs: rs[1])
            runs = []
            for row, s in pairs:
                if runs and runs[-1][0] + 2 * runs[-1][2] == s \
                        and runs[-1][1] + runs[-1][2] == row:
                    runs[-1][2] += 1
                else:
                    runs.append([s, row, 1])
            for s0, r0, n_ in runs:
                nc.scalar.activation(
                    out=_ap(expS[:], s0 * 64, [[2560, 25], [128, n_], [1, 64]]),
                    in_=_ap(pcs[:], 32 * 384 + (r0 - g0) * 64,
                            [[384, 25], [64, n_], [1, 64]]),
                    func=AF.Exp, bias=bco_sb[32:57], scale=1.0)
            lo, hi = max(g0, 2), min(g0 + 6, 34)
            if lo < hi:
                nc.vector.tensor_scalar(
                    out=_ap(offS[:], oslot(lo - 2) * 64, [[2048, 8], [128, hi - lo], [1, 64]]),
                    in0=_ap(pcs[:], (lo - g0) * 64, [[384, 8], [64, hi - lo], [1, 64]]),
                    scalar1=bco_sb[0:8], scalar2=None, op0=OP.add)

        # ---- 3. PE transposes to 128-partition layout (p = 64*jh + w) ----
        # offsets first: pairs (m, m+16) -> deltT2 [128, 16, 8]
        deltT2 = work.tile([128, 128], BF16)
        pt2 = psc.tile([128, 512], F32, name='pt2', tag='pcs')
        for m in range(16):
            nc.tensor.transpose(pt2[:, m * 8:m * 8 + 8],
                                _ap(offS[:], 2 * m * 64, [[2048, 8], [1, 128]]),
                                id_sb[0:8, 0:8])
        nc.scalar.activation(out=deltT2[:], in_=pt2[:, 0:128], func=AF.Copy, scale=1.0)
        # exp: slot pair (2r, 2r+1) -> expT2 [128, 20 hl, 25 k]; hl = mask row - 16jh
        expT2 = work.tile([128, 20, 25], F32)
        pt = psc.tile([128, 512], F32, name='pt', tag='pcs')
        for r in range(20):
            nc.tensor.transpose(pt[:, r * 25:r * 25 + 25],
                                _ap(expS[:], 2 * r * 64, [[2560, 25], [1, 128]]),
                                id_sb[0:25, 0:25])
        nc.scalar.activation(out=expT2[:], in_=pt[:, 0:500], func=AF.Copy, scale=1.0)

        # ---- 4. W9 bilinear-indicator weights [128, 16m, 4u] ----
        def dview(chbase):
            return _ap(deltT2[:], chbase, [[128, 128], [8, 16], [1, 4]])

        def wt(nm):
            return work.tile([128, 64], BF16, name=nm)

        t1, t2 = wt('t1'), wt('t2')
        gxc, x0r, wxt, omwx, x1r = wt('gxc'), wt('x0r'), wt('wxt'), wt('omwx'), wt('x1r')
        gyc, y0r, wyt, omwy, y1r = wt('gyc'), wt('y0r'), wt('wyt'), wt('omwy'), wt('y1r')
        ia, ib = wt('ia'), wt('ib')
        cwx = work.tile([128, 3, 64], BF16)
        rwy = work.tile([128, 3, 64], BF16)
        W9b = work.tile([128, 9 * 64], BF16)

        hrow_bc = _ap(hrow_sb[:], 0, [[16, 128], [1, 16], [0, 4]])
        y63_bc = _ap(y63_sb[:], 0, [[16, 128], [1, 16], [0, 4]])

        def r4(ap):
            return _ap(ap, 0, [[64, 128], [4, 16], [1, 4]])

        nc.vector.tensor_scalar(out=t1[:], in0=dview(0), scalar1=wvec_sb[:], scalar2=None, op0=OP.add)
        nc.vector.tensor_scalar(out=t2[:], in0=t1[:], scalar1=0.0, scalar2=63.0, op0=OP.max, op1=OP.min)
        nc.vector.tensor_scalar(out=gxc[:], in0=t2[:], scalar1=wvec_sb[:], scalar2=None, op0=OP.subtract)
        nc.vector.tensor_scalar(out=x0r[:], in0=gxc[:], scalar1=0.0, scalar2=-1.0, op0=OP.is_lt, op1=OP.mult)
        nc.vector.tensor_tensor(out=wxt[:], in0=gxc[:], in1=x0r[:], op=OP.subtract)
        nc.vector.tensor_scalar(out=omwx[:], in0=wxt[:], scalar1=-1.0, scalar2=1.0, op0=OP.mult, op1=OP.add)
        nc.vector.tensor_scalar(out=x1r[:], in0=x0r[:], scalar1=1.0, scalar2=w63_sb[:], op0=OP.add, op1=OP.min)

        nc.vector.tensor_tensor(out=r4(t1[:]), in0=dview(4), in1=hrow_bc, op=OP.add)
        nc.vector.tensor_scalar(out=t2[:], in0=t1[:], scalar1=0.0, scalar2=63.0, op0=OP.max, op1=OP.min)
        nc.vector.tensor_tensor(out=r4(gyc[:]), in0=r4(t2[:]), in1=hrow_bc, op=OP.subtract)
        nc.vector.tensor_scalar(out=y0r[:], in0=gyc[:], scalar1=0.0, scalar2=-1.0, op0=OP.is_lt, op1=OP.mult)
        nc.vector.tensor_tensor(out=wyt[:], in0=gyc[:], in1=y0r[:], op=OP.subtract)
        nc.vector.tensor_scalar(out=omwy[:], in0=wyt[:], scalar1=-1.0, scalar2=1.0, op0=OP.mult, op1=OP.add)
        nc.vector.tensor_scalar(out=t1[:], in0=y0r[:], scalar1=1.0, scalar2=None, op0=OP.add)
        nc.vector.tensor_tensor(out=r4(y1r[:]), in0=r4(t1[:]), in1=y63_bc, op=OP.min)

        for i, e in enumerate((-1.0, 0.0, 1.0)):
            nc.vector.tensor_scalar(out=ia[:], in0=x0r[:], scalar1=e, scalar2=None, op0=OP.is_equal)
            nc.vector.tensor_scalar(out=ib[:], in0=x1r[:], scalar1=e, scalar2=None, op0=OP.is_equal)
            nc.vector.tensor_tensor(out=ia[:], in0=ia[:], in1=omwx[:], op=OP.mult)
            nc.vector.tensor_tensor(out=ib[:], in0=ib[:], in1=wxt[:], op=OP.mult)
            nc.vector.tensor_tensor(out=cwx[:, i, :], in0=ia[:], in1=ib[:], op=OP.add)
            nc.vector.tensor_scalar(out=ia[:], in0=y0r[:], scalar1=e, scalar2=None, op0=OP.is_equal)
            nc.vector.tensor_scalar(out=ib[:], in0=y1r[:], scalar1=e, scalar2=None, op0=OP.is_equal)
            nc.vector.tensor_tensor(out=ia[:], in0=ia[:], in1=omwy[:], op=OP.mult)
            nc.vector.tensor_tensor(out=ib[:], in0=ib[:], in1=wyt[:], op=OP.mult)
            nc.vector.tensor_tensor(out=rwy[:, i, :], in0=ia[:], in1=ib[:], op=OP.add)
        for iy in range(3):
            for ix in range(3):
                nc.vector.tensor_tensor(
                    out=_ap(W9b[:], (iy * 3 + ix) * 64, [[9 * 64, 128], [1, 64]]),
                    in0=rwy[:, iy, :], in1=cwx[:, ix, :], op=OP.mult)

        # ---- 5. softmax (u-expanded) + shifted variants ----
        sumT = work.tile([128, 20], F32)
        nc.vector.tensor_reduce(out=sumT[:], in_=expT2[:], axis=mybir.AxisListType.X, op=OP.add)
        recT = work.tile([128, 20], F32)
        nc.vector.reciprocal(out=recT[:], in_=sumT[:])
        msm4 = work.tile([128, 2000], BF16)   # [128, 20 hl, 25 k, 4 u]
        nc.vector.tensor_tensor(
            out=_ap(msm4[:], 0, [[2000, 128], [100, 20], [4, 25], [1, 4]]),
            in0=_ap(expT2[:], 0, [[500, 128], [25, 20], [1, 25], [0, 4]]),
            in1=_ap(recT[:], 0, [[20, 128], [1, 20], [0, 25], [0, 4]]), op=OP.mult)
        for blk in range(2):
            b0 = 64 * blk * 2000
            nc.sync.dma_start(out=_ap(msm4_p1[:], b0, [[2000, 63], [1, 2000]]),
                              in_=_ap(msm4[:], b0 + 2000, [[2000, 63], [1, 2000]]))
            nc.scalar.dma_start(out=_ap(msm4_m1[:], b0 + 2000, [[2000, 63], [1, 2000]]),
                                in_=_ap(msm4[:], b0, [[2000, 63], [1, 2000]]))

        # ---- 5-9 software-pipelined by m-groups ----
        kernc = work.tile([128, 1600], BF16)   # [128, 16 m, 25 k, 4 u] (k is kx-major)
        tmp = work.tile([128, 1600], BF16)
        data_all = work.tile([128, 16, 100], BF16)
        msm_by_ex = {-1: msm4_m1, 0: msm4, 1: msm4_p1}
        NG = 4
        GM = 16 // NG
        # center/ex=0 terms first so the msm4 shift DMAs can complete in their shadow
        E_ORDER = [(0, 1), (1, 1), (2, 1), (0, 0), (1, 0), (2, 0), (0, 2), (1, 2), (2, 2)]

        def emit_asm(G):
            kv = _ap(kernc[:], GM * G * 100, [[1600, 128], [100, GM], [4, 25], [1, 4]])
            tv = _ap(tmp[:], GM * G * 100, [[1600, 128], [100, GM], [4, 25], [1, 4]])
            for n_, (iy, ix) in enumerate(E_ORDER):
                ey, ex = iy - 1, ix - 1
                mv = _ap(msm_by_ex[ex][:], (2 + ey + GM * G) * 100,
                         [[2000, 128], [100, GM], [4, 25], [1, 4]])
                wv = _ap(W9b[:], (iy * 3 + ix) * 64 + GM * G * 4,
                         [[9 * 64, 128], [4, GM], [0, 25], [1, 4]])
                if n_ == 0:
                    nc.vector.tensor_tensor(out=kv, in0=wv, in1=mv, op=OP.mult)
                else:
                    nc.vector.tensor_tensor(out=tv, in0=wv, in1=mv, op=OP.mult)
                    nc.vector.tensor_tensor(out=kv, in0=kv, in1=tv, op=OP.add)

        def emit_dmas(G):
            # kbf[d][p] = kernc[p+d] within each 64-block (block edges stay zero)
            for d in (-2, -1, 1, 2):
                cnt = 64 - abs(d)
                eng = nc.sync if d > 0 else nc.scalar
                for blk in range(2):
                    b0 = 64 * blk * 1600
                    oo = b0 + (-d if d < 0 else 0) * 1600 + GM * G * 100
                    io = b0 + (d if d > 0 else 0) * 1600 + GM * G * 100
                    eng.dma_start(
                        out=_ap(kbf[d][:], oo, [[1600, cnt], [1, GM * 100]]),
                        in_=_ap(kernc[:], io, [[1600, cnt], [1, GM * 100]]))

        def emit_prep(G):
            # data_all[p, m, b*20+ky*4+u] = kernc[p+b-2, m, kx=4-b, ky, u]
            for b in range(5):
                src = kernc if b == 2 else kbf[b - 2]
                nc.scalar.activation(
                    out=_ap(data_all[:], GM * G * 100 + b * 20, [[1600, 128], [100, GM], [1, 20]]),
                    in_=_ap(src[:], GM * G * 100 + (4 - b) * 20, [[1600, 128], [100, GM], [1, 20]]),
                    func=AF.Copy, scale=1.0)

        def emit_pairs(G):
            for m in range(GM * G, GM * G + GM):
                banded1 = band.tile([128, 1536], BF16, name=f'band1_{m}', tag='band1')
                banded2 = band.tile([128, 1024], BF16, name=f'band2_{m}', tag='band2')
                nc.gpsimd.local_scatter(out_ap=banded1[:], data_ap=data_all[:, m, :],
                                        idxs_ap=idx1_sb[:], channels=128, num_elems=1536, num_idxs=100)
                nc.gpsimd.local_scatter(out_ap=banded2[:], data_ap=data_all[:, m, :],
                                        idxs_ap=idx2_sb[:], channels=128, num_elems=1024, num_idxs=100)
                for ch in range(2):
                    pcs = psc.tile([128, 512], F32, name=f'pcs_{m}_{ch}', tag='pcs')
                    for ki in range(5):
                        lhsT = _ap(xT2_sb[:], (m + ki) * 256 + ch * 128, [[20 * 256, 128], [1, 128]])
                        rhs = banded1[:, ki * 512:ki * 512 + 512] if ki < 3 \
                            else banded2[:, (ki - 3) * 512:(ki - 3) * 512 + 512]
                        nc.tensor.matmul(pcs[:], lhsT, rhs, start=(ki == 0), stop=(ki == 4))
                    rb = rowp.tile([128, 512], BF16, name=f'rb_{m}_{ch}', tag='rb')
                    nc.scalar.activation(out=rb[:], in_=pcs[:], func=AF.Copy, scale=1.0)
                    nc.sync.dma_start(
                        out=_ap(outp[:], ch * 128 * 8192 + 4 * m * 128,
                                [[8192, 128], [128, 4], [1, 128]]),
                        in_=rb[:])

        emit_asm(0)
        emit_dmas(0)
        emit_prep(0)
        for G in range(NG):
            if G + 1 < NG:
                emit_asm(G + 1)
                emit_dmas(G + 1)
                emit_prep(G + 1)
            emit_pairs(G)
    nc.finalize()
    return nc


_PROGRAM = None
_SCAT = build_scatter_tables()


def _get_program():
    global _PROGRAM
    if _PROGRAM is None:
        _PROGRAM = build_program()
    return _PROGRAM


def _prep_core_inputs(inputs, n, s):
    bf = np.float16
    x = np.asarray(inputs['x'][n], np.float32)
    h0 = 32 * s
    xw = np.zeros((C, 38, W), np.float32)
    for i, g in enumerate(range(h0 - 3, h0 + 35)):
        if 0 <= g < H:
            xw[:, i] = x[:, g]
    xwin = np.ascontiguousarray(xw.reshape(2, 128, 38 * 64)).astype(bf)
    xT2 = np.zeros((128, 20, C), np.float32)
    for jh in range(2):
        base = h0 + 16 * jh - 2
        for i in range(20):
            g = base + i
            if 0 <= g < H:
                xT2[64 * jh:64 * jh + 64, i] = x[:, g].T
    xT2 = np.ascontiguousarray(xT2.reshape(128, 20 * 256)).astype(bf)
    w_comp = np.asarray(inputs['w_comp'], np.float32)[:, :, 0, 0]
    wc = np.zeros((2, 128, 64), np.float32)
    for cg in range(2):
        wc[cg] = w_comp[:, cg * 128:(cg + 1) * 128].T
    wc = np.ascontiguousarray(wc.transpose(1, 0, 2).reshape(128, 2 * 64)).astype(bf)
    w_ker = np.asarray(inputs['w_ker'], np.float32)
    w_off = np.asarray(inputs['w_off'], np.float32)
    # mask channels permuted kx-major: new k = kx*5+ky holds w_ker[ky*5+kx]
    kperm = np.array([(k % 5) * 5 + k // 5 for k in range(25)])
    wk = np.zeros((9, 64, 57), np.float32)
    for t in range(9):
        wk[t, :, 0:8] = w_off[:, :, t // 3, t % 3].T
        wk[t, :, 32:57] = w_ker[kperm, :, t // 3, t % 3].T
    wk = np.ascontiguousarray(wk.transpose(1, 0, 2).reshape(64, 9 * 57)).astype(bf)
    bcov = np.zeros((57, 1), np.float32)
    bcov[0:8, 0] = np.asarray(inputs['b_off'], np.float32)
    bcov[32:57, 0] = np.asarray(inputs['b_ker'], np.float32)[kperm]
    idx1, idx2 = _SCAT
    pp = np.arange(128, dtype=np.float32)
    hr = (h0 + 16.0 * (pp // 64))[:, None] + np.arange(16, dtype=np.float32)[None, :]
    return {
        'xwin': xwin, 'xT2': xT2, 'wc': wc, 'wk': wk, 'bco': bcov,
        'bcomp': np.asarray(inputs['b_comp'], np.float32).reshape(64, 1),
        'wvec': (pp % 64).reshape(128, 1),
        'w63': (63.0 - pp % 64).reshape(128, 1),
        'hrow': np.ascontiguousarray(hr),
        'y63': np.ascontiguousarray(63.0 - hr),
        'ident': np.eye(128, dtype=np.float32),
        'idx1': idx1, 'idx2': idx2,
        'zed': np.zeros((2, 3600), np.float16),
    }


def kernel(**inputs):
    nc = _get_program()
    core_ids = list(range(8))
    in_maps = [_prep_core_inputs(inputs, cid // 2, cid % 2) for cid in core_ids]
    res = run_bass_kernel_spmd(nc, in_maps, core_ids)
    out = np.zeros((N, C, 128, 128), np.float32)
    for cid in core_ids:
        n, s = cid // 2, cid % 2
        op = np.asarray(res.results[cid]['outp']).reshape(256, 64, 128)
        out[n, :, s::2] = op
    return out


if __name__ == '__main__':
    d = np.load('/root/problem/ref_io.npz')
    inp = {k: d[k] for k in ('x', 'w_comp', 'b_comp', 'w_ker', 'b_ker', 'w_off', 'b_off')}
    out = kernel(**inp)
    ref = d['out']
    err = np.abs(out - ref).max()
    print('max abs err:', err, 'rel:', err / np.abs(ref).max())


# revision 28
# speedup vs baseline: 1.2351x; 1.0907x over previous
"""Trainium2 Bass kernel for nn_DLUPack (CARAFE-style dynamic upsampling).

Sharding: 8 cores = (batch n in [0,4)) x (output-row-parity s in {0,1});
core (n, s) computes low-res rows hh in [32s, 32s+32) -> all parity-s output rows.

Reference output mapping (its reshape scrambles positions):
  ref[n, c, 2y+i, 2x+j] = sum_k patches[c, hh, ww, k] * kern[hh, ww, k, u]
  with hh = 32s + 16jh + m:  row r = 8m + 2(ww//16) + s, col = 8*(ww%16) + 2u + jh.

Device pipeline per core (128-partition mid-section: p = 64*jh + w):
  1. compressor 1x1 conv (PE, fp16) -> cx [64, 38, 66]
  2. offset+mask 3x3 convs (9 accumulated MMs) -> psum [57, .]: off ch 0-7, mask ch 32-56
  3. exp in ACT evac -> expS [25, 36, 64]; PE-transpose row-pairs (r, r+16)
     -> expT2 [128 = jh*64+w, 20 hl, 25 k]; softmax via free-dim reduce
  4. offset PE-transpose pairs (m, m+16) -> deltT2 [128, 16, 8]; W9 weights (DVE)
  5. kernc assembly [128, 16m, 25k, 4u]: 9 broadcast-multiply terms (DVE)
  6. kernc block-shifted +-1,+-2 via SBUF-SBUF DMA (800B runs); prep slices -> data_all
  7. per pair m: 2 local_scatter (GPSIMD) -> banded [128, 3*512 + 2*512]
  8. carafe: 5 accumulated MMs [128,128]x[128,512] per (pair, c-half) -> psum [128,512]
  9. ACT evac (fp16) -> DMA out 4 contiguous output rows (host converts to f32)
"""
import sys
import numpy as np

sys.path.insert(0, '/opt/trn_rl_repo')

import ml_dtypes  # noqa: E402
from contextlib import ExitStack  # noqa: E402

import concourse.bass as bass  # noqa: E402
import concourse.tile as tile  # noqa: E402
from concourse import mybir, bacc  # noqa: E402
from concourse.bass_utils import run_bass_kernel_spmd  # noqa: E402

F32 = mybir.dt.float32
BF16 = mybir.dt.float16  # NOTE: fp16 (better mantissa), name kept for brevity
I16 = mybir.dt.int16
AF = mybir.ActivationFunctionType
OP = mybir.AluOpType

N, C, H, W = 4, 256, 64, 64


def _ap(base, off_elems, dims):
    return bass.AP(tensor=base.tensor, offset=base.offset + off_elems, ap=[list(d) for d in dims])


def build_scatter_tables():
    idx1 = -np.ones((128, 100), np.int16)
    idx2 = -np.ones((128, 100), np.int16)
    for p in range(128):
        jh, wpp = p // 64, p % 64
        for b in range(5):
            w = wpp + b - 2
            if not (0 <= w < 64):
                continue
            q, wl = w // 16, w % 16
            for ki in range(5):
                for u in range(4):
                    col = q * 128 + 8 * wl + 2 * u + jh
                    qidx = (b * 5 + ki) * 4 + u
                    if ki < 3:
                        idx1[p, qidx] = ki * 512 + col
                    else:
                        idx2[p, qidx] = (ki - 3) * 512 + col
    return idx1, idx2


def build_program():
    nc = bacc.Bacc(None, target_bir_lowering=False, debug=True)

    xwin = nc.declare_dram_parameter('xwin', [2, 128, 38 * 64], BF16, isOutput=False)
    xT2 = nc.declare_dram_parameter('xT2', [128, 20 * 256], BF16, isOutput=False)
    wc = nc.declare_dram_parameter('wc', [128, 2 * 64], BF16, isOutput=False)
    wk = nc.declare_dram_parameter('wk', [64, 9 * 57], BF16, isOutput=False)
    bco = nc.declare_dram_parameter('bco', [57, 1], F32, isOutput=False)
    bcomp = nc.declare_dram_parameter('bcomp', [64, 1], F32, isOutput=False)
    wvec = nc.declare_dram_parameter('wvec', [128, 1], F32, isOutput=False)
    w63 = nc.declare_dram_parameter('w63', [128, 1], F32, isOutput=False)
    hrow = nc.declare_dram_parameter('hrow', [128, 16], F32, isOutput=False)
    y63 = nc.declare_dram_parameter('y63', [128, 16], F32, isOutput=False)
    ident = nc.declare_dram_parameter('ident', [128, 128], F32, isOutput=False)
    idx1 = nc.declare_dram_parameter('idx1', [128, 100], I16, isOutput=False)
    idx2 = nc.declare_dram_parameter('idx2', [128, 100], I16, isOutput=False)
    zed = nc.declare_dram_parameter('zed', [2, 3600], BF16, isOutput=False)
    outp = nc.declare_dram_parameter('outp', [256, 64 * 128], BF16, isOutput=True)

    with tile.TileContext(nc) as tc, ExitStack() as ctx:
        sing = ctx.enter_context(tc.tile_pool(name='sing', bufs=1))
        work = ctx.enter_context(tc.tile_pool(name='work', bufs=1))
        band = ctx.enter_context(tc.tile_pool(name='band', bufs=4))
        rowp = ctx.enter_context(tc.tile_pool(name='rowp', bufs=4))
        psum = ctx.enter_context(tc.psum_pool(name='ps', bufs=2))
        psc = ctx.enter_context(tc.psum_pool(name='psc', bufs=4))

        def load(shape, dtype, src, eng):
            t = sing.tile(shape, dtype, name=f'ld_{src.tensor.name if hasattr(src, "tensor") else id(src)}')
            eng.dma_start(out=t[:], in_=src[:])
            return t

        # critical small loads first so warm-up + compressor can begin
        id_sb = load([128, 128], F32, ident, nc.sync)
        wc_sb = load([128, 2, 64], BF16, wc, nc.sync)
        xwin_sb = sing.tile([128, 2, 38 * 64], BF16)
        for grp in range(5):
            g0 = grp * 8
            rows = min(8, 38 - g0)
            for cg_ in range(2):
                nc.sync.dma_start(
                    out=_ap(xwin_sb[:], cg_ * 2432 + g0 * 64,
                            [[4864, 128], [1, rows * 64]]),
                    in_=_ap(xwin[:], cg_ * 128 * 2432 + g0 * 64,
                            [[2432, 128], [1, rows * 64]]))
        wk_sb = load([64, 9, 57], BF16, wk, nc.scalar)
        xT2_sb = load([128, 20 * 256], BF16, xT2, nc.scalar)
        bco_sb = load([57, 1], F32, bco, nc.gpsimd)
        bcomp_sb = load([64, 1], F32, bcomp, nc.gpsimd)
        wvec_sb = load([128, 1], F32, wvec, nc.gpsimd)
        w63_sb = load([128, 1], F32, w63, nc.gpsimd)
        hrow_sb = load([128, 16], F32, hrow, nc.gpsimd)
        y63_sb = load([128, 16], F32, y63, nc.gpsimd)
        idx1_sb = load([128, 100], I16, idx1, nc.gpsimd)
        idx2_sb = load([128, 100], I16, idx2, nc.gpsimd)

        # msm4 shifted variants; block-edge partitions zeroed via tiny DMAs from DRAM zeros
        msm4_p1 = work.tile([128, 500], BF16)   # msm4_p1[p] = msm4[p+1] (within 64-block)
        msm4_m1 = work.tile([128, 500], BF16)   # msm4_m1[p] = msm4[p-1]
        for blk in range(2):
            nc.scalar.dma_start(out=_ap(msm4_p1[:], (64 * blk + 63) * 500, [[500, 1], [1, 500]]),
                                in_=_ap(zed[:], 0, [[3600, 1], [1, 500]]))
            nc.scalar.dma_start(out=_ap(msm4_m1[:], 64 * blk * 500, [[500, 1], [1, 500]]),
                                in_=_ap(zed[:], 0, [[3600, 1], [1, 500]]))

        # kernc block-shift buffers; memset once so block-edge partitions stay zero
        kbf = {}
        for d in (-2, -1, 1, 2):
            kbf[d] = work.tile([128, 1600], BF16, name=f'kbf{d}')
            nc.vector.memset(kbf[d][:], 0.0)

        # PE warm-up: keep TensorE busy during input-DMA wait (DVFS ramp)
        pw = psc.tile([128, 512], F32, name='pcs_warm', tag='pcs')
        for _ in range(32):
            nc.tensor.matmul(pw[0:64, 0:64], id_sb[:, 0:64], id_sb[:, 0:64], start=True, stop=True)

        # ---- 1. compressor ----
        cx_sb = work.tile([64, 38, 66], BF16)
        nc.vector.memset(_ap(cx_sb[:], 0, [[38 * 66, 64], [66, 38], [1, 1]]), 0.0)
        nc.vector.memset(_ap(cx_sb[:], 65, [[38 * 66, 64], [66, 38], [1, 1]]), 0.0)
        for grp in range(5):
            g0 = grp * 8
            rows = min(8, 38 - g0)
            nn = rows * 64
            pcs = psum.tile([64, 512], F32)
            for cg in range(2):
                nc.tensor.matmul(pcs[:, :nn], wc_sb[:, cg, :],
                                 xwin_sb[:, cg, g0 * 64:g0 * 64 + nn],
                                 start=(cg == 0), stop=(cg == 1))
            nc.scalar.activation(
                out=_ap(cx_sb[:], g0 * 66 + 1, [[38 * 66, 64], [66, rows], [1, 64]]),
                in_=_ap(pcs[:], 0, [[512, 64], [64, rows], [1, 64]]),
                func=AF.Identity, bias=bcomp_sb[:], scale=1.0)

        # ---- 2. offset+mask convs ----
        # expS2/offS2 store interleaved row pairs: slot 2r = row r, slot 2r+1 = row r+16
        # so the [*,128] PE transposes read contiguous pairs. expS rows 16-19 duplicated.
        expS = work.tile([25, 40, 64], F32)
        offS = work.tile([8, 32, 64], F32)

        def eslot(row):  # primary slot for mask row
            return 2 * row if row < 20 else 2 * (row - 16) + 1

        def oslot(row):  # slot for offset row (no duplication)
            return 2 * row if row < 16 else 2 * (row - 16) + 1

        for grp in range(6):
            g0 = grp * 6
            nn = 6 * 64
            pcs = psum.tile([57, 384], F32)
            for t in range(9):
                dy, dx = t // 3, t % 3
                rhs = _ap(cx_sb[:], (g0 + dy) * 66 + dx, [[38 * 66, 64], [66, 6], [1, 64]])
                nc.tensor.matmul(pcs[:, :nn], wk_sb[:, t, :], rhs,
                                 start=(t == 0), stop=(t == 8))
            # evac mask rows to interleaved slots, one ACT per maximal stride-2 run
            pairs = []
            for row in range(g0, g0 + 6):
                if row < 20:
                    pairs.append((row, 2 * row))
                if row >= 16:
                    pairs.append((row, 2 * (row - 16) + 1))
            pairs.sort(key=lambda rs: rs[1])
            runs = []
            for row, s in pairs:
                if runs and runs[-1][0] + 2 * runs[-1][2] == s \
                        and runs[-1][1] + runs[-1][2] == row:
                    runs[-1][2] += 1
                else:
                    runs.append([s, row, 1])
            for s0, r0, n_ in runs:
                nc.scalar.activation(
                    out=_ap(expS[:], s0 * 64, [[2560, 25], [128, n_], [1, 64]]),
                    in_=_ap(pcs[:], 32 * 384 + (r0 - g0) * 64,
                            [[384, 25], [64, n_], [1, 64]]),
                    func=AF.Exp, bias=bco_sb[32:57], scale=1.0)
            lo, hi = max(g0, 2), min(g0 + 6, 34)
            if lo < hi:
                nc.vector.tensor_scalar(
                    out=_ap(offS[:], oslot(lo - 2) * 64, [[2048, 8], [128, hi - lo], [1, 64]]),
                    in0=_ap(pcs[:], (lo - g0) * 64, [[384, 8], [64, hi - lo], [1, 64]]),
                    scalar1=bco_sb[0:8], scalar2=None, op0=OP.add)

        # ---- 3. PE transposes to 128-partition layout (p = 64*jh + w) ----
        # offsets first: pairs (m, m+16) -> deltT2 [128, 16, 8]
        deltT2 = work.tile([128, 128], BF16)
        pt2 = psc.tile([128, 512], F32, name='pt2', tag='pcs')
        for m in range(16):
            nc.tensor.transpose(pt2[:, m * 8:m * 8 + 8],
                                _ap(offS[:], 2 * m * 64, [[2048, 8], [1, 128]]),
                                id_sb[0:8, 0:8])
        nc.scalar.activation(out=deltT2[:], in_=pt2[:, 0:128], func=AF.Copy, scale=1.0)
        # exp: slot pair (2r, 2r+1) -> expT2 [128, 20 hl, 25 k]; hl = mask row - 16jh
        expT2 = work.tile([128, 20, 25], F32)
        pt = psc.tile([128, 512], F32, name='pt', tag='pcs')
        for r in range(20):
            nc.tensor.transpose(pt[:, r * 25:r * 25 + 25],
                                _ap(expS[:], 2 * r * 64, [[2560, 25], [1, 128]]),
                                id_sb[0:25, 0:25])
        nc.scalar.activation(out=expT2[:], in_=pt[:, 0:500], func=AF.Copy, scale=1.0)

        # ---- 4. W9 bilinear-indicator weights [128, 16m, 4u] ----
        def dview(chbase):
            return _ap(deltT2[:], chbase, [[128, 128], [8, 16], [1, 4]])

        def wt(nm):
            return work.tile([128, 64], BF16, name=nm)

        t1, t2 = wt('t1'), wt('t2')
        gxc, x0r, wxt, omwx, x1r = wt('gxc'), wt('x0r'), wt('wxt'), wt('omwx'), wt('x1r')
        gyc, y0r, wyt, omwy, y1r = wt('gyc'), wt('y0r'), wt('wyt'), wt('omwy'), wt('y1r')
        ia, ib = wt('ia'), wt('ib')
        cwx = work.tile([128, 3, 64], BF16)
        rwy = work.tile([128, 3, 64], BF16)
        W9b = work.tile([128, 9 * 64], BF16)

        hrow_bc = _ap(hrow_sb[:], 0, [[16, 128], [1, 16], [0, 4]])
        y63_bc = _ap(y63_sb[:], 0, [[16, 128], [1, 16], [0, 4]])

        def r4(ap):
            return _ap(ap, 0, [[64, 128], [4, 16], [1, 4]])

        nc.vector.tensor_scalar(out=t1[:], in0=dview(0), scalar1=wvec_sb[:], scalar2=None, op0=OP.add)
        nc.vector.tensor_scalar(out=t2[:], in0=t1[:], scalar1=0.0, scalar2=63.0, op0=OP.max, op1=OP.min)
        nc.vector.tensor_scalar(out=gxc[:], in0=t2[:], scalar1=wvec_sb[:], scalar2=None, op0=OP.subtract)
        nc.vector.tensor_scalar(out=x0r[:], in0=gxc[:], scalar1=0.0, scalar2=-1.0, op0=OP.is_lt, op1=OP.mult)
        nc.vector.tensor_tensor(out=wxt[:], in0=gxc[:], in1=x0r[:], op=OP.subtract)
        nc.vector.tensor_scalar(out=omwx[:], in0=wxt[:], scalar1=-1.0, scalar2=1.0, op0=OP.mult, op1=OP.add)
        nc.vector.tensor_scalar(out=x1r[:], in0=x0r[:], scalar1=1.0, scalar2=w63_sb[:], op0=OP.add, op1=OP.min)

        nc.vector.tensor_tensor(out=r4(t1[:]), in0=dview(4), in1=hrow_bc, op=OP.add)
        nc.vector.tensor_scalar(out=t2[:], in0=t1[:], scalar1=0.0, scalar2=63.0, op0=OP.max, op1=OP.min)
        nc.vector.tensor_tensor(out=r4(gyc[:]), in0=r4(t2[:]), in1=hrow_bc, op=OP.subtract)
        nc.vector.tensor_scalar(out=y0r[:], in0=gyc[:], scalar1=0.0, scalar2=-1.0, op0=OP.is_lt, op1=OP.mult)
        nc.vector.tensor_tensor(out=wyt[:], in0=gyc[:], in1=y0r[:], op=OP.subtract)
        nc.vector.tensor_scalar(out=omwy[:], in0=wyt[:], scalar1=-1.0, scalar2=1.0, op0=OP.mult, op1=OP.add)
        nc.vector.tensor_scalar(out=t1[:], in0=y0r[:], scalar1=1.0, scalar2=None, op0=OP.add)
        nc.vector.tensor_tensor(out=r4(y1r[:]), in0=r4(t1[:]), in1=y63_bc, op=OP.min)

        # x0r/y0r in {-1,0}, x1r/y1r in {0,1} always, so the e=-1 weight has only the
        # "0-side" term and e=+1 only the "1-side" term.
        nc.vector.tensor_scalar(out=ia[:], in0=x0r[:], scalar1=-1.0, scalar2=None, op0=OP.is_equal)
        nc.vector.tensor_tensor(out=cwx[:, 0, :], in0=ia[:], in1=omwx[:], op=OP.mult)
        nc.vector.tensor_scalar(out=ia[:], in0=x1r[:], scalar1=1.0, scalar2=None, op0=OP.is_equal)
        nc.vector.tensor_tensor(out=cwx[:, 2, :], in0=ia[:], in1=wxt[:], op=OP.mult)
        nc.vector.tensor_scalar(out=ia[:], in0=x0r[:], scalar1=0.0, scalar2=None, op0=OP.is_equal)
        nc.vector.tensor_scalar(out=ib[:], in0=x1r[:], scalar1=0.0, scalar2=None, op0=OP.is_equal)
        nc.vector.tensor_tensor(out=ia[:], in0=ia[:], in1=omwx[:], op=OP.mult)
        nc.vector.tensor_tensor(out=ib[:], in0=ib[:], in1=wxt[:], op=OP.mult)
        nc.vector.tensor_tensor(out=cwx[:, 1, :], in0=ia[:], in1=ib[:], op=OP.add)
        nc.vector.tensor_scalar(out=ia[:], in0=y0r[:], scalar1=-1.0, scalar2=None, op0=OP.is_equal)
        nc.vector.tensor_tensor(out=rwy[:, 0, :], in0=ia[:], in1=omwy[:], op=OP.mult)
        nc.vector.tensor_scalar(out=ia[:], in0=y1r[:], scalar1=1.0, scalar2=None, op0=OP.is_equal)
        nc.vector.tensor_tensor(out=rwy[:, 2, :], in0=ia[:], in1=wyt[:], op=OP.mult)
        nc.vector.tensor_scalar(out=ia[:], in0=y0r[:], scalar1=0.0, scalar2=None, op0=OP.is_equal)
        nc.vector.tensor_scalar(out=ib[:], in0=y1r[:], scalar1=0.0, scalar2=None, op0=OP.is_equal)
        nc.vector.tensor_tensor(out=ia[:], in0=ia[:], in1=omwy[:], op=OP.mult)
        nc.vector.tensor_tensor(out=ib[:], in0=ib[:], in1=wyt[:], op=OP.mult)
        nc.vector.tensor_tensor(out=rwy[:, 1, :], in0=ia[:], in1=ib[:], op=OP.add)
        for iy in range(3):
            for ix in range(3):
                nc.vector.tensor_tensor(
                    out=_ap(W9b[:], (iy * 3 + ix) * 64, [[9 * 64, 128], [1, 64]]),
                    in0=rwy[:, iy, :], in1=cwx[:, ix, :], op=OP.mult)

        # ---- 5. softmax (u-expanded) + shifted variants ----
        sumT = work.tile([128, 20], F32)
        nc.vector.tensor_reduce(out=sumT[:], in_=expT2[:], axis=mybir.AxisListType.X, op=OP.add)
        recT = work.tile([128, 20], F32)
        nc.vector.reciprocal(out=recT[:], in_=sumT[:])
        msm4 = work.tile([128, 500], BF16)   # [128, 20 hl, 25 k]; u broadcast via 0-stride
        nc.vector.tensor_tensor(
            out=_ap(msm4[:], 0, [[500, 128], [25, 20], [1, 25]]),
            in0=_ap(expT2[:], 0, [[500, 128], [25, 20], [1, 25]]),
            in1=_ap(recT[:], 0, [[20, 128], [1, 20], [0, 25]]), op=OP.mult)
        for blk in range(2):
            b0 = 64 * blk * 500
            nc.sync.dma_start(out=_ap(msm4_p1[:], b0, [[500, 63], [1, 500]]),
                              in_=_ap(msm4[:], b0 + 500, [[500, 63], [1, 500]]))
            nc.scalar.dma_start(out=_ap(msm4_m1[:], b0 + 500, [[500, 63], [1, 500]]),
                                in_=_ap(msm4[:], b0, [[500, 63], [1, 500]]))

        # ---- 5-9 software-pipelined by m-groups ----
        kernc = work.tile([128, 1600], BF16)   # [128, 16 m, 25 k, 4 u] (k is kx-major)
        tmpA = work.tile([128, 1600], BF16)
        tmpB = work.tile([128, 1600], BF16)
        data_all = work.tile([128, 16, 100], BF16)
        msm_by_ex = {-1: msm4_m1, 0: msm4, 1: msm4_p1}
        NG = 4
        GM = 16 // NG
        # center/ex=0 terms first so the msm4 shift DMAs can complete in their shadow
        E_ORDER = [(0, 1), (1, 1), (2, 1), (0, 0), (1, 0), (2, 0), (0, 2), (1, 2), (2, 2)]

        def emit_asm(G):
            # interleave M0 M1 M2 A1 M3 A2 ... (two tmp tiles) to keep RAW deps
            # >= 2 ops apart and hide the DVE pipeline latency
            def kv():
                return _ap(kernc[:], GM * G * 100, [[1600, 128], [100, GM], [4, 25], [1, 4]])

            def tv(t_):
                return _ap(t_[:], GM * G * 100, [[1600, 128], [100, GM], [4, 25], [1, 4]])

            def term(n_):
                iy, ix = E_ORDER[n_]
                ey, ex = iy - 1, ix - 1
                mv = _ap(msm_by_ex[ex][:], (2 + ey + GM * G) * 25,
                         [[500, 128], [25, GM], [1, 25], [0, 4]])
                wv = _ap(W9b[:], (iy * 3 + ix) * 64 + GM * G * 4,
                         [[9 * 64, 128], [4, GM], [0, 25], [1, 4]])
                return mv, wv

            mv, wv = term(0)
            nc.vector.tensor_tensor(out=kv(), in0=wv, in1=mv, op=OP.mult)  # M0
            tms = [tmpA, tmpB]
            for j in (1, 2):                                               # M1 M2
                mv, wv = term(j)
                nc.vector.tensor_tensor(out=tv(tms[j - 1]), in0=wv, in1=mv, op=OP.mult)
            for j in range(1, 9):                                          # A_j (+ M_{j+2})
                nc.vector.tensor_tensor(out=kv(), in0=kv(), in1=tv(tms[(j - 1) % 2]), op=OP.add)
                if j + 2 <= 8:
                    mv, wv = term(j + 2)
                    nc.vector.tensor_tensor(out=tv(tms[(j + 1) % 2]), in0=wv, in1=mv, op=OP.mult)

        def emit_dmas(G):
            # kbf[d][p] = kernc[p+d] within each 64-block (block edges stay zero)
            for d in (-2, -1, 1, 2):
                cnt = 64 - abs(d)
                eng = nc.sync if d > 0 else nc.scalar
                for blk in range(2):
                    b0 = 64 * blk * 1600
                    oo = b0 + (-d if d < 0 else 0) * 1600 + GM * G * 100
                    io = b0 + (d if d > 0 else 0) * 1600 + GM * G * 100
                    eng.dma_start(
                        out=_ap(kbf[d][:], oo, [[1600, cnt], [1, GM * 100]]),
                        in_=_ap(kernc[:], io, [[1600, cnt], [1, GM * 100]]))

        def emit_prep(G):
            # data_all[p, m, b*20+ky*4+u] = kernc[p+b-2, m, kx=4-b, ky, u]
            for b in (2, 1, 3, 0, 4):   # b=2 reads kernc directly (no DMA dependency)
                src = kernc if b == 2 else kbf[b - 2]
                nc.vector.tensor_copy(
                    out=_ap(data_all[:], GM * G * 100 + b * 20, [[1600, 128], [100, GM], [1, 20]]),
                    in_=_ap(src[:], GM * G * 100 + (4 - b) * 20, [[1600, 128], [100, GM], [1, 20]]))

        def emit_pairs(G):
            for m in range(GM * G, GM * G + GM):
                banded1 = band.tile([128, 1536], BF16, name=f'band1_{m}', tag='band1')
                banded2 = band.tile([128, 1024], BF16, name=f'band2_{m}', tag='band2')
                nc.gpsimd.local_scatter(out_ap=banded1[:], data_ap=data_all[:, m, :],
                                        idxs_ap=idx1_sb[:], channels=128, num_elems=1536, num_idxs=100)
                nc.gpsimd.local_scatter(out_ap=banded2[:], data_ap=data_all[:, m, :],
                                        idxs_ap=idx2_sb[:], channels=128, num_elems=1024, num_idxs=100)
                for ch in range(2):
                    pcs = psc.tile([128, 512], F32, name=f'pcs_{m}_{ch}', tag='pcs')
                    for ki in range(5):
                        lhsT = _ap(xT2_sb[:], (m + ki) * 256 + ch * 128, [[20 * 256, 128], [1, 128]])
                        rhs = banded1[:, ki * 512:ki * 512 + 512] if ki < 3 \
                            else banded2[:, (ki - 3) * 512:(ki - 3) * 512 + 512]
                        nc.tensor.matmul(pcs[:], lhsT, rhs, start=(ki == 0), stop=(ki == 4))
                    rb = rowp.tile([128, 512], BF16, name=f'rb_{m}_{ch}', tag='rb')
                    nc.scalar.activation(out=rb[:], in_=pcs[:], func=AF.Copy, scale=1.0)
                    nc.sync.dma_start(
                        out=_ap(outp[:], ch * 128 * 8192 + 4 * m * 128,
                                [[8192, 128], [128, 4], [1, 128]]),
                        in_=rb[:])

        emit_asm(0)
        emit_dmas(0)
        emit_prep(0)
        for G in range(NG):
            if G + 1 < NG:
                emit_asm(G + 1)
                emit_dmas(G + 1)
                emit_prep(G + 1)
            emit_pairs(G)
    nc.finalize()
    return nc


_PROGRAM = None
_SCAT = build_scatter_tables()


def _get_program():
    global _PROGRAM
    if _PROGRAM is None:
        _PROGRAM = build_program()
    return _PROGRAM


def _prep_core_inputs(inputs, n, s):
    bf = np.float16
    x = np.asarray(inputs['x'][n], np.float32)
    h0 = 32 * s
    xw = np.zeros((C, 38, W), np.float32)
    for i, g in enumerate(range(h0 - 3, h0 + 35)):
        if 0 <= g < H:
            xw[:, i] = x[:, g]
    xwin = np.ascontiguousarray(xw.reshape(2, 128, 38 * 64)).astype(bf)
    xT2 = np.zeros((128, 20, C), np.float32)
    for jh in range(2):
        base = h0 + 16 * jh - 2
        for i in range(20):
            g = base + i
            if 0 <= g < H:
                xT2[64 * jh:64 * jh + 64, i] = x[:, g].T
    xT2 = np.ascontiguousarray(xT2.reshape(128, 20 * 256)).astype(bf)
    w_comp = np.asarray(inputs['w_comp'], np.float32)[:, :, 0, 0]
    wc = np.zeros((2, 128, 64), np.float32)
    for cg in range(2):
        wc[cg] = w_comp[:, cg * 128:(cg + 1) * 128].T
    wc = np.ascontiguousarray(wc.transpose(1, 0, 2).reshape(128, 2 * 64)).astype(bf)
    w_ker = np.asarray(inputs['w_ker'], np.float32)
    w_off = np.asarray(inputs['w_off'], np.float32)
    # mask channels permuted kx-major: new k = kx*5+ky holds w_ker[ky*5+kx]
    kperm = np.array([(k % 5) * 5 + k // 5 for k in range(25)])
    wk = np.zeros((9, 64, 57), np.float32)
    for t in range(9):
        wk[t, :, 0:8] = w_off[:, :, t // 3, t % 3].T
        wk[t, :, 32:57] = w_ker[kperm, :, t // 3, t % 3].T
    wk = np.ascontiguousarray(wk.transpose(1, 0, 2).reshape(64, 9 * 57)).astype(bf)
    bcov = np.zeros((57, 1), np.float32)
    bcov[0:8, 0] = np.asarray(inputs['b_off'], np.float32)
    bcov[32:57, 0] = np.asarray(inputs['b_ker'], np.float32)[kperm]
    idx1, idx2 = _SCAT
    pp = np.arange(128, dtype=np.float32)
    hr = (h0 + 16.0 * (pp // 64))[:, None] + np.arange(16, dtype=np.float32)[None, :]
    return {
        'xwin': xwin, 'xT2': xT2, 'wc': wc, 'wk': wk, 'bco': bcov,
        'bcomp': np.asarray(inputs['b_comp'], np.float32).reshape(64, 1),
        'wvec': (pp % 64).reshape(128, 1),
        'w63': (63.0 - pp % 64).reshape(128, 1),
        'hrow': np.ascontiguousarray(hr),
        'y63': np.ascontiguousarray(63.0 - hr),
        'ident': np.eye(128, dtype=np.float32),
        'idx1': idx1, 'idx2': idx2,
        'zed': np.zeros((2, 3600), np.float16),
    }


def kernel(**inputs):
    nc = _get_program()
    core_ids = list(range(8))
    in_maps = [_prep_core_inputs(inputs, cid // 2, cid % 2) for cid in core_ids]
    res = run_bass_kernel_spmd(nc, in_maps, core_ids)
    out = np.zeros((N, C, 128, 128), np.float32)
    for cid in core_ids:
        n, s = cid // 2, cid % 2
        op = np.asarray(res.results[cid]['outp']).reshape(256, 64, 128)
        out[n, :, s::2] = op
    return out


if __name__ == '__main__':
    d = np.load('/root/problem/ref_io.npz')
    inp = {k: d[k] for k in ('x', 'w_comp', 'b_comp', 'w_ker', 'b_ker', 'w_off', 'b_off')}
    out = kernel(**inp)
    ref = d['out']
    err = np.abs(out - ref).max()
    print('max abs err:', err, 'rel:', err / np.abs(ref).max())
